# revision 1
# baseline (speedup 1.0000x reference)
"""Trainium2 Bass kernel for nn_CrossAttentionBlock (B=4, T=4096, C=512, H=8,
INNER=2048, NIN=2) on 8 NeuronCores.

Sharding: core c handles batch b=c//2, token half h=c%2 (2048 tokens each).
All per-token math is local; the only cross-core coupling is the linear-
attention context (ctx = k^T v, [H,64,64] per batch) and k_sum, reduced with
pair-wise AllReduces (cores 2b and 2b+1).

On-chip layout: the residual stream and all dense math are feature-major
([128 features, 512 tokens] fp32r tiles) so every projection/FFN matmul runs
with a 512-wide moving dim at full PE rate. k/v are produced token-major for
the ctx contraction. LN stats and partition-broadcasts are done with small
ones/selector matmuls on the PE. The softmax-q normalization and the
linear-attention D^-1 are folded into one reciprocal + broadcast pass using
unnormalized E = exp(qp):  out = E/S + sum_i (E @ ctx_i) / G_i with
G_i = sum_d E * ksum_i (the 1e-8 eps is ~1e-6 relative here and dropped).
"""
import os
import numpy as np

import concourse.bass as bass
import concourse.tile as tile
from concourse import mybir
from concourse.vector_clock import ScopedClock
from concourse.bass_utils import run_bass_kernel_spmd

F32 = mybir.dt.float32
F32R = mybir.dt.float32r
AF = mybir.ActivationFunctionType
OP = mybir.AluOpType

B, T, C, H, D, INNER, NIN = 4, 4096, 512, 8, 64, 2048, 2
N_CORES = 8
NTOK = 2048          # tokens per core
CHUNK = 512          # tokens per chunk
NCH = NTOK // CHUNK  # 4 chunks
FT = C // 128        # 4 feature tiles
IT = INNER // 128    # 16 inner tiles
LN_EPS = 1e-5
GROUPS = [[0, 1], [2, 3], [4, 5], [6, 7]]

_split_counter = [0]


def _split_multi_waits(nc):
    """This walrus build only supports one sync-wait per instruction; move
    extra waits onto same-engine NoOps placed immediately before."""
    for f in nc.m.functions:
        for blk in f.blocks:
            out = []
            changed = False
            for inst in blk.instructions:
                si = inst.sync_info
                if si is not None and si.on_wait and len(si.on_wait) > 1:
                    waits = list(si.on_wait)
                    for w in waits[:-1]:
                        _split_counter[0] += 1
                        nop = mybir.InstNoOp(
                            name=f"I-waitsplit-{_split_counter[0]}", ins=[], outs=[]
                        )
                        nop.engine = inst.engine
                        nop.sync_info = mybir.SyncInfo(on_wait=[w], on_update=[])
                        out.append(nop)
                    si.on_wait = waits[-1:]
                    inst.sync_info = si
                    changed = True
                out.append(inst)
            if changed:
                blk.instructions = out


class _TC(tile.TileContext):
    def _drain_and_barrier(self, tick_clock, wait_clock):
        drain_inst = self.nc.sync.drain()
        wait_clock.add_sem_waits(
            drain_inst.ins, ScopedClock({None: tick_clock.global_clock})
        )
        si = drain_inst.ins.sync_info
        if si is not None and si.on_wait and len(si.on_wait) > 1:
            waits = list(si.on_wait)
            si.on_wait = waits[:1]
            drain_inst.ins.sync_info = si
            for i in range(1, len(waits)):
                extra = self.nc.sync.drain()
                esi = extra.ins.sync_info
                if esi is None:
                    extra.ins.sync_info = mybir.SyncInfo(
                        on_wait=waits[i : i + 1], on_update=[]
                    )
                else:
                    esi.on_wait = waits[i : i + 1]
                    extra.ins.sync_info = esi
        self.nc.all_engine_barrier()
        assert self.sems is not None
        popped = self.nc._tile_sem_poison_stack.pop()
        assert popped is self._sem_poison
        self.nc.clear_and_free_semaphores(list(self.sems.allocated().values()))
        self.nc.all_engine_barrier()


def _build_program(split=True):
    nc = bass.Bass("TRN2", target_bir_lowering=False, debug=False, num_devices=N_CORES)
    I = {}

    def di(name, shape):
        I[name] = nc.dram_tensor(name, list(shape), F32, kind="ExternalInput").ap()

    di("xT", [C, NTOK])
    di("ysT", [NIN, C, NTOK])
    for w in ["wq", "wo", "saq", "sak", "sav", "sao"]:
        di(w, [C, C])
    di("wk", [NIN, C, C])
    di("wv", [NIN, C, C])
    di("f1w1", [C, INNER])
    di("f1w2", [INNER, C])
    di("f2w1", [C, INNER])
    di("f2w2", [INNER, C])
    for bname in ["bq_c", "bo_c", "saq_c", "sao_c", "f1b2_c", "f2b2_c"]:
        di(bname, [128, FT])
    di("f1b1_c", [128, IT])
    di("f2b1_c", [128, IT])
    di("bk_r", [NIN, 1, C])
    di("bv_r", [NIN, 1, C])
    di("sak_r", [1, C])
    di("sav_r", [1, C])
    for lname in ["ln1", "ln3", "ln4", "ln5"]:
        di(lname + "_g", [128, FT])
        di(lname + "_b", [128, FT])
    di("ln2_g", [NIN, 128, FT])
    di("ln2_b", [NIN, 128, FT])
    di("ones_c", [1, 128])
    di("ones_r", [128, 1])
    di("sgbase", [FT, 128, 24])
    di("sel8", [FT, 8, 128])
    di("zz", [128, 128])

    out_t = nc.dram_tensor("outT", [C, NTOK], F32, kind="ExternalOutput").ap()

    with _TC(nc) as tc:
        _Emitter(nc, tc, I, out_t).run()
    if split:
        _split_multi_waits(nc)
    return nc


class _Emitter:
    def __init__(self, nc, tc, I, out_t):
        self.nc, self.tc, self.I, self.out_t = nc, tc, I, out_t

    # ---------------- helpers ----------------
    def layer_norm(self, x_tiles, gt, bt):
        nc = self.nc
        sum_ps = self.p_stats.tile([1, CHUNK], F32, tag="stats", name="stats")
        for k in range(FT):
            nc.tensor.matmul(sum_ps, self.ONESR, x_tiles[k],
                             start=(k == 0), stop=(k == FT - 1))
        srow = self.rows.tile([1, CHUNK], F32, tag="rows", name="rows")
        nc.vector.tensor_copy(srow, sum_ps)
        xsq = []
        for k in range(FT):
            sq = self.lntmp.tile([128, CHUNK], F32R, tag="xsq", name="xsq")
            nc.scalar.activation(out=sq, in_=x_tiles[k].bitcast(F32),
                                 func=AF.Square)
            xsq.append(sq)
        sq_ps = self.p_stats.tile([1, CHUNK], F32, tag="stats", name="stats")
        for k in range(FT):
            nc.tensor.matmul(sq_ps, self.ONESR, xsq[k],
                             start=(k == 0), stop=(k == FT - 1))
        qrow = self.rows.tile([1, CHUNK], F32, tag="rows", name="rows")
        nc.vector.tensor_copy(qrow, sq_ps)
        mrow = self.rows.tile([1, CHUNK], F32, tag="rows", name="rows")
        nc.vector.tensor_scalar(out=mrow, in0=srow, scalar1=1.0 / C,
                                scalar2=None, op0=OP.mult)
        m2 = self.rows.tile([1, CHUNK], F32, tag="rows", name="rows")
        nc.vector.tensor_tensor(out=m2, in0=mrow, in1=mrow, op=OP.mult)
        v1 = self.rows.tile([1, CHUNK], F32, tag="rows", name="rows")
        nc.vector.tensor_scalar(out=v1, in0=qrow, scalar1=1.0 / C,
                                scalar2=None, op0=OP.mult)
        var = self.rows.tile([1, CHUNK], F32, tag="rows", name="rows")
        nc.vector.tensor_tensor(out=var, in0=v1, in1=m2, op=OP.subtract)
        sq_ = self.rows.tile([1, CHUNK], F32, tag="rows", name="rows")
        nc.scalar.activation(out=sq_, in_=var, func=AF.Sqrt, bias=self.EPS,
                             scale=1.0)
        arow = self.rows.tile([1, CHUNK], F32R, tag="rows", name="rows")
        with nc.allow_low_precision(reason="fp32r feeds matmul"):
            nc.vector.reciprocal(out=arow, in_=sq_)
        negm = self.rows.tile([1, CHUNK], F32, tag="rows", name="rows")
        nc.vector.tensor_scalar(out=negm, in0=srow, scalar1=-1.0 / C,
                                scalar2=None, op0=OP.mult)
        brow = self.rows.tile([1, CHUNK], F32R, tag="rows", name="rows")
        with nc.allow_low_precision(reason="fp32r feeds matmul"):
            nc.vector.tensor_tensor(out=brow, in0=negm, in1=arow.bitcast(F32),
                                    op=OP.mult)
        a_ps = self.p_bc.tile([128, CHUNK], F32, tag="bc", name="bc")
        nc.tensor.matmul(a_ps, self.ONESC, arow, start=True, stop=True)
        b_ps = self.p_bc.tile([128, CHUNK], F32, tag="bc", name="bc")
        nc.tensor.matmul(b_ps, self.ONESC, brow, start=True, stop=True)
        bsb = self.lntmp.tile([128, CHUNK], F32, tag="bsb", name="bsb")
        nc.scalar.activation(out=bsb, in_=b_ps, func=AF.Copy, bias=0.0,
                             scale=1.0)
        asb = self.lntmp.tile([128, CHUNK], F32, tag="asb", name="asb")
        nc.scalar.activation(out=asb, in_=a_ps, func=AF.Copy, bias=0.0,
                             scale=1.0)
        outs = []
        for k in range(FT):
            t1 = self.lntmp.tile([128, CHUNK], F32, tag="lnt", name="lnt")
            nc.vector.tensor_tensor(out=t1, in0=x_tiles[k].bitcast(F32),
                                    in1=asb, op=OP.mult)
            t2 = self.lntmp.tile([128, CHUNK], F32, tag="lnt", name="lnt")
            nc.vector.tensor_tensor(out=t2, in0=t1, in1=bsb, op=OP.add)
            xk = self.xnp.tile([128, CHUNK], F32R, tag="xn", name="xn")
            nc.scalar.activation(out=xk, in_=t2, func=AF.Identity,
                                 bias=bt[:, k : k + 1], scale=gt[:, k : k + 1])
            outs.append(xk)
        return outs

    def proj_fm_psum(self, w_tiles, xn_tiles, m):
        ps = self.p_mm.tile([128, CHUNK], F32, tag="mm", name="mm")
        for k in range(FT):
            self.nc.tensor.matmul(ps, w_tiles[k][:, 128 * m : 128 * (m + 1)],
                                  xn_tiles[k], start=(k == 0),
                                  stop=(k == FT - 1))
        return ps

    def proj_tm_psum(self, w_tiles, xn_tiles, t, bias_row):
        ps = self.p_mm.tile([128, CHUNK], F32, tag="mm", name="mm")
        self.nc.tensor.matmul(ps, self.ONESC, bias_row, start=True, stop=False)
        for k in range(FT):
            self.nc.tensor.matmul(ps, xn_tiles[k][:, 128 * t : 128 * (t + 1)],
                                  w_tiles[k], start=False, stop=(k == FT - 1))
        return ps

    def softmax_token_major(self, kps, kvp, ketmp, smallp):
        nc = self.nc
        kE = ketmp.tile([128, C], F32, tag="kE", name="kE")
        nc.scalar.activation(out=kE, in_=kps, func=AF.Exp)
        ssum = smallp.tile([128, H], F32, tag="ssum", name="ssum")
        nc.vector.tensor_reduce(
            out=ssum, in_=kE.rearrange("p (h d) -> p h d", d=D),
            axis=mybir.AxisListType.X, op=OP.add)
        rsum = smallp.tile([128, H], F32, tag="rsum", name="rsum")
        nc.vector.reciprocal(out=rsum, in_=ssum)
        kn = kvp.tile([128, C], F32R, tag="kn", name="kn")
        with nc.allow_low_precision(reason="fp32r feeds matmul"):
            for h in range(H):
                nc.vector.tensor_scalar(
                    out=kn[:, D * h : D * (h + 1)],
                    in0=kE[:, D * h : D * (h + 1)],
                    scalar1=rsum[:, h : h + 1], scalar2=None, op0=OP.mult)
        return kn

    def load_w512(self, ap, pool, tag):
        tiles = []
        for k in range(FT):
            t = pool.tile([128, C], F32R, tag=f"{tag}{k}", name=f"{tag}{k}")
            self.nc.sync.dma_start(
                out=t, in_=ap[128 * k : 128 * (k + 1), :].bitcast(F32R))
            tiles.append(t)
        return tiles

    def attn_front(self, Xin, wq_ap, bq_cols, lng, lnb):
        """LN + q-projection + exp for all chunks -> E tiles."""
        nc = self.nc
        E = [[None] * FT for _ in range(NCH)]
        with self.tc.tile_pool(name="w_q", bufs=1) as w_q:
            WQ = self.load_w512(wq_ap, w_q, "wq")
            for ch in range(NCH):
                xn = self.layer_norm(Xin[ch], lng, lnb)
                for m in range(FT):
                    ps = self.proj_fm_psum(WQ, xn, m)
                    e = self.epool.tile([128, CHUNK], F32R, tag="E", name="E")
                    nc.scalar.activation(out=e, in_=ps, func=AF.Exp,
                                         bias=bq_cols[:, m : m + 1], scale=1.0)
                    E[ch][m] = e
        return E

    def attn_back(self, Xin, E, cc_out, n_in, wo_ap, bo_cols, sg_w, new_resid):
        """SG/G reciprocals, broadcasts, block-diag apply, assembly, wo
        projection + residual. cc_out: DRAM tile ([n_in,65,C] or [65,C])."""
        nc, tc, I = self.nc, self.tc, self.I
        Xout = [[None] * FT for _ in range(NCH)]
        cc = (lambda i: cc_out[i]) if n_in > 1 else (lambda i: cc_out)
        with tc.tile_pool(name=f"w_{sg_w}", bufs=1) as w_o, \
             tc.tile_pool(name=f"as_{sg_w}", bufs=1) as attn_s, \
             tc.tile_pool(name=f"tmp_{sg_w}", bufs=4) as atmp, \
             tc.tile_pool(name=f"rec_{sg_w}", bufs=3) as recp:
            WO = self.load_w512(wo_ap, w_o, "wo")
            ncols = 8 + 8 * n_in
            SGT = []
            for c in range(FT):
                sg = attn_s.tile([128, ncols], F32R, tag=f"sgt{c}", name=f"sgt{c}")
                nc.sync.dma_start(
                    out=sg, in_=I["sgbase"][c][:, 0:ncols].bitcast(F32R))
                for i in range(n_in):
                    col = 8 + 8 * i + 2 * c
                    nc.gpsimd.dma_start(
                        out=sg[0:D, col : col + 1],
                        in_=cc(i)[D, 128 * c : 128 * c + D].rearrange(
                            "(p o) -> p o", o=1).bitcast(F32R))
                    nc.gpsimd.dma_start(
                        out=sg[D:128, col + 1 : col + 2],
                        in_=cc(i)[D, 128 * c + D : 128 * (c + 1)].rearrange(
                            "(p o) -> p o", o=1).bitcast(F32R))
                SGT.append(sg)
            BD = [[None] * FT for _ in range(n_in)]
            for i in range(n_in):
                for c in range(FT):
                    bd = attn_s.tile([128, 128], F32R, tag=f"bd{i}_{c}", name=f"bd{i}_{c}")
                    nc.sync.dma_start(out=bd, in_=I["zz"].bitcast(F32R))
                    nc.gpsimd.dma_start(
                        out=bd[0:D, 0:D],
                        in_=cc(i)[0:D, (2 * c) * D : (2 * c + 1) * D].bitcast(F32R))
                    nc.gpsimd.dma_start(
                        out=bd[D:128, D:128],
                        in_=cc(i)[0:D, (2 * c + 1) * D : (2 * c + 2) * D].bitcast(F32R))
                    BD[i][c] = bd

            for ch in range(NCH):
                recs = []
                for j in range(1 + n_in):
                    gps = self.p_stats.tile([8, CHUNK], F32, tag="stats", name="stats")
                    for c in range(FT):
                        nc.tensor.matmul(gps, SGT[c][:, 8 * j : 8 * (j + 1)],
                                         E[ch][c], start=(c == 0),
                                         stop=(c == FT - 1))
                    r = recp.tile([8, CHUNK], F32, tag="rec", name="rec")
                    nc.vector.reciprocal(out=r, in_=gps)
                    rr = recp.tile([8, CHUNK], F32R, tag="recr", name="recr")
                    nc.scalar.activation(out=rr, in_=r, func=AF.Copy, bias=0.0,
                                         scale=1.0)
                    recs.append(rr)
                outc = []
                for c in range(FT):
                    aps = []
                    gsb = []
                    for i in range(n_in):
                        a = self.p_mm.tile([128, CHUNK], F32, tag="mm", name="mm")
                        nc.tensor.matmul(a, BD[i][c], E[ch][c], start=True,
                                         stop=True)
                        asb_ = atmp.tile([128, CHUNK], F32, tag="apb", name="apb")
                        nc.scalar.activation(out=asb_, in_=a, func=AF.Copy,
                                             bias=0.0, scale=1.0)
                        aps.append(asb_)
                        gb = self.p_bc.tile([128, CHUNK], F32, tag="bc", name="bc")
                        nc.tensor.matmul(gb, self.SEL8[c], recs[1 + i],
                                         start=True, stop=True)
                        gs = atmp.tile([128, CHUNK], F32, tag="gbs", name="gbs")
                        nc.scalar.activation(out=gs, in_=gb, func=AF.Copy,
                                             bias=0.0, scale=1.0)
                        gsb.append(gs)
                    sb = self.p_bc.tile([128, CHUNK], F32, tag="bc", name="bc")
                    nc.tensor.matmul(sb, self.SEL8[c], recs[0], start=True,
                                     stop=True)
                    ssb = atmp.tile([128, CHUNK], F32, tag="gbs", name="gbs")
                    nc.scalar.activation(out=ssb, in_=sb, func=AF.Copy,
                                         bias=0.0, scale=1.0)
                    acc = atmp.tile([128, CHUNK], F32, tag="asm", name="asm")
                    nc.vector.tensor_tensor(out=acc, in0=E[ch][c].bitcast(F32),
                                            in1=ssb, op=OP.mult)
                    for i in range(n_in):
                        ai = atmp.tile([128, CHUNK], F32, tag="asm", name="asm")
                        nc.vector.tensor_tensor(out=ai, in0=gsb[i], in1=aps[i],
                                                op=OP.mult)
                        last = (i == n_in - 1)
                        nxt = self.xnp.tile([128, CHUNK], F32R, tag="xn", name="xn") if last \
                            else atmp.tile([128, CHUNK], F32, tag="asm", name="asm")
                        with nc.allow_low_precision(reason="fp32r feeds matmul"):
                            nc.vector.tensor_tensor(
                                out=nxt, in0=acc.bitcast(F32), in1=ai, op=OP.add)
                        acc = nxt
                    outc.append(acc)
                for m in range(FT):
                    wps = self.proj_fm_psum(WO, outc, m)
                    tt = self.wotp.tile([128, CHUNK], F32, tag="wot", name="wot")
                    nc.scalar.activation(out=tt, in_=wps, func=AF.Identity,
                                         bias=bo_cols[:, m : m + 1], scale=1.0)
                    xo = new_resid()
                    with nc.allow_low_precision(reason="fp32r feeds matmul"):
                        nc.vector.tensor_tensor(out=xo,
                                                in0=Xin[ch][m].bitcast(F32),
                                                in1=tt, op=OP.add)
                    Xout[ch][m] = xo
        return Xout

    def ffn(self, Xin, w1name, w2name, B1, B2, lng, lnb):
        nc, tc, I = self.nc, self.tc, self.I
        Xout = [[None] * FT for _ in range(NCH)]
        with tc.tile_pool(name=w1name, bufs=1) as w1p, \
             tc.tile_pool(name=w2name + "s", bufs=6) as w2p, \
             tc.tile_pool(name=w1name + "h", bufs=4) as hp, \
             tc.tile_pool(name=w1name + "p", bufs=4, space="PSUM") as p_ffn:
            W1 = []
            for k in range(FT):
                t = w1p.tile([128, INNER], F32R, tag=f"w1_{k}", name=f"w1_{k}")
                nc.sync.dma_start(
                    out=t, in_=I[w1name][128 * k : 128 * (k + 1), :].bitcast(F32R))
                W1.append(t)
            for ch in range(NCH):
                xn = self.layer_norm(Xin[ch], lng, lnb)
                ops = [p_ffn.tile([128, CHUNK], F32, tag="ffn", name="ffn")
                       for _ in range(FT)]
                for k in range(IT):
                    hps = self.p_mm.tile([128, CHUNK], F32, tag="mm", name="mm")
                    for c in range(FT):
                        nc.tensor.matmul(hps, W1[c][:, 128 * k : 128 * (k + 1)],
                                         xn[c], start=(c == 0),
                                         stop=(c == FT - 1))
                    h = hp.tile([128, CHUNK], F32R, tag="h", name="h")
                    nc.scalar.activation(out=h, in_=hps, func=AF.Gelu_apprx_tanh,
                                         bias=B1[:, k : k + 1], scale=1.0)
                    w2t = w2p.tile([128, C], F32R, tag="w2s", name="w2s")
                    nc.sync.dma_start(
                        out=w2t,
                        in_=I[w2name][128 * k : 128 * (k + 1), :].bitcast(F32R))
                    for m in range(FT):
                        nc.tensor.matmul(ops[m],
                                         w2t[:, 128 * m : 128 * (m + 1)], h,
                                         start=(k == 0), stop=(k == IT - 1))
                for m in range(FT):
                    tt = self.wotp.tile([128, CHUNK], F32, tag="wot", name="wot")
                    nc.scalar.activation(out=tt, in_=ops[m], func=AF.Identity,
                                         bias=B2[:, m : m + 1], scale=1.0)
                    xo = self.resid.tile([128, CHUNK], F32R, tag="resid", name="resid")
                    with nc.allow_low_precision(reason="fp32r feeds matmul"):
                        nc.vector.tensor_tensor(out=xo,
                                                in0=Xin[ch][m].bitcast(F32),
                                                in1=tt, op=OP.add)
                    Xout[ch][m] = xo
        return Xout

    # ---------------- main ----------------
    def run(self):
        nc, tc, I = self.nc, self.tc, self.I
        from contextlib import ExitStack

        with ExitStack() as ctx:
            const = ctx.enter_context(tc.tile_pool(name="const", bufs=1))
            self.resid = ctx.enter_context(tc.tile_pool(name="resid", bufs=20))
            self.epool = ctx.enter_context(tc.tile_pool(name="E", bufs=16))
            self.xnp = ctx.enter_context(tc.tile_pool(name="xn", bufs=5))
            self.rows = ctx.enter_context(tc.tile_pool(name="rows", bufs=8))
            self.lntmp = ctx.enter_context(tc.tile_pool(name="lntmp", bufs=3))
            self.wotp = ctx.enter_context(tc.tile_pool(name="wot", bufs=3))
            dram = ctx.enter_context(tc.tile_pool(name="dram", bufs=1,
                                                  space="DRAM"))
            self.p_mm = ctx.enter_context(
                tc.tile_pool(name="p_mm", bufs=2, space="PSUM"))
            self.p_stats = ctx.enter_context(
                tc.tile_pool(name="p_stats", bufs=1, space="PSUM"))
            self.p_bc = ctx.enter_context(
                tc.tile_pool(name="p_bc", bufs=1, space="PSUM"))

            # ---------------- constants ----------------
            self.EPS = const.tile([1, 1], F32, tag="eps", name="eps")
            nc.vector.memset(self.EPS, LN_EPS)
            self.ONESC = const.tile([1, 128], F32R, tag="onesc", name="onesc")
            nc.sync.dma_start(out=self.ONESC, in_=I["ones_c"].bitcast(F32R))
            self.ONESR = const.tile([128, 1], F32R, tag="onesr", name="onesr")
            nc.sync.dma_start(out=self.ONESR, in_=I["ones_r"].bitcast(F32R))
            self.SEL8 = []
            for c in range(FT):
                s = const.tile([8, 128], F32R, tag=f"sel8_{c}", name=f"sel8_{c}")
                nc.sync.dma_start(out=s, in_=I["sel8"][c].bitcast(F32R))
                self.SEL8.append(s)

            def cols_tile(name, nt):
                t = const.tile([128, nt], F32, tag=name)
                nc.sync.dma_start(out=t, in_=I[name])
                return t

            BQ = cols_tile("bq_c", FT)
            BO = cols_tile("bo_c", FT)
            SAQ = cols_tile("saq_c", FT)
            SAO = cols_tile("sao_c", FT)
            F1B1 = cols_tile("f1b1_c", IT)
            F1B2 = cols_tile("f1b2_c", FT)
            F2B1 = cols_tile("f2b1_c", IT)
            F2B2 = cols_tile("f2b2_c", FT)
            LNG, LNB = {}, {}
            for lname in ["ln1", "ln3", "ln4", "ln5"]:
                LNG[lname] = cols_tile(lname + "_g", FT)
                LNB[lname] = cols_tile(lname + "_b", FT)
            for i in range(NIN):
                g = const.tile([128, FT], F32, tag=f"ln2g{i}", name=f"ln2g{i}")
                nc.sync.dma_start(out=g, in_=I["ln2_g"][i])
                b = const.tile([128, FT], F32, tag=f"ln2b{i}", name=f"ln2b{i}")
                nc.sync.dma_start(out=b, in_=I["ln2_b"][i])
                LNG[f"ln2_{i}"], LNB[f"ln2_{i}"] = g, b

            def row_tile(apslice, tag):
                t = const.tile([1, C], F32R, tag=tag)
                nc.sync.dma_start(out=t, in_=apslice.bitcast(F32R))
                return t

            BKR = [row_tile(I["bk_r"][i], f"bkr{i}") for i in range(NIN)]
            BVR = [row_tile(I["bv_r"][i], f"bvr{i}") for i in range(NIN)]
            SAKR = row_tile(I["sak_r"], "sakr")
            SAVR = row_tile(I["sav_r"], "savr")

            # ---------------- residual load ----------------
            X = [[self.resid.tile([128, CHUNK], F32R, tag="resid", name="resid")
                  for _ in range(FT)] for _ in range(NCH)]
            for ch in range(NCH):
                for c in range(FT):
                    nc.sync.dma_start(
                        out=X[ch][c],
                        in_=I["xT"][128 * c : 128 * (c + 1),
                                    CHUNK * ch : CHUNK * (ch + 1)].bitcast(F32R))

            # ============ phase A: CA front ============
            E = self.attn_front(X, I["wq"], BQ, LNG["ln1"], LNB["ln1"])
            cc_in = dram.tile([NIN, D + 1, C], F32, tag="cc_ca_in", name="cc_ca_in")
            cc_out = dram.tile([NIN, D + 1, C], F32, tag="cc_ca_out", name="cc_ca_out")
            with tc.tile_pool(name="w_kv", bufs=1) as w_kv, \
                 tc.tile_pool(name="ysp", bufs=4) as ysp, \
                 tc.tile_pool(name="kvp", bufs=2) as kvp, \
                 tc.tile_pool(name="kep", bufs=2) as kep, \
                 tc.tile_pool(name="smallp", bufs=4) as smallp, \
                 tc.tile_pool(name="ctxsb", bufs=1) as ctxsbp, \
                 tc.tile_pool(name="p_ctx", bufs=2, space="PSUM") as p_ctx, \
                 tc.tile_pool(name="p_ks", bufs=2, space="PSUM") as p_ks:
                WK = [self.load_w512(I["wk"][i], w_kv, f"wk{i}")
                      for i in range(NIN)]
                WV = [self.load_w512(I["wv"][i], w_kv, f"wv{i}")
                      for i in range(NIN)]
                CTXA = [ctxsbp.tile([D, C], F32, tag=f"ctxacc{i}",
                                    name=f"ctxacc{i}") for i in range(NIN)]
                KSA = [ctxsbp.tile([1, C], F32, tag=f"ksacc{i}",
                                   name=f"ksacc{i}") for i in range(NIN)]
                for ch in range(NCH):
                    for i in range(NIN):
                        yt = []
                        for c in range(FT):
                            y = ysp.tile([128, CHUNK], F32R, tag="ys", name="ys")
                            nc.sync.dma_start(
                                out=y,
                                in_=I["ysT"][i, 128 * c : 128 * (c + 1),
                                             CHUNK * ch : CHUNK * (ch + 1)
                                             ].bitcast(F32R))
                            yt.append(y)
                        yn = self.layer_norm(yt, LNG[f"ln2_{i}"],
                                             LNB[f"ln2_{i}"])
                        ctx_ps = p_ctx.tile([D, C], F32, tag="ctx", name="ctx")
                        ks_ps = p_ks.tile([1, C], F32, tag="ks", name="ks")
                        for t in range(FT):
                            kps = self.proj_tm_psum(WK[i], yn, t, BKR[i])
                            kn = self.softmax_token_major(kps, kvp, kep, smallp)
                            vps = self.proj_tm_psum(WV[i], yn, t, BVR[i])
                            vn = kvp.tile([128, C], F32R, tag="vn", name="vn")
                            nc.scalar.activation(out=vn, in_=vps, func=AF.Copy,
                                                 bias=0.0, scale=1.0)
                            for h in range(H):
                                nc.tensor.matmul(
                                    ctx_ps[:, D * h : D * (h + 1)],
                                    kn[:, D * h : D * (h + 1)],
                                    vn[:, D * h : D * (h + 1)],
                                    start=(t == 0 and h == 0),
                                    stop=(t == FT - 1 and h == H - 1))
                            nc.tensor.matmul(ks_ps, self.ONESR, kn,
                                             start=(t == 0),
                                             stop=(t == FT - 1))
                        if ch == 0:
                            nc.vector.tensor_copy(CTXA[i], ctx_ps)
                            nc.vector.tensor_copy(KSA[i], ks_ps)
                        else:
                            nc.vector.tensor_tensor(out=CTXA[i], in0=CTXA[i],
                                                    in1=ctx_ps, op=OP.add)
                            nc.vector.tensor_tensor(out=KSA[i], in0=KSA[i],
                                                    in1=ks_ps, op=OP.add)
                for i in range(NIN):
                    nc.sync.dma_start(out=cc_in[i, 0:D, :], in_=CTXA[i])
                    nc.sync.dma_start(out=cc_in[i, D : D + 1, :], in_=KSA[i])
            nc.gpsimd.collective_compute(
                "AllReduce", OP.add, replica_groups=GROUPS,
                ins=[cc_in[:].opt()], outs=[cc_out[:].opt()])

            # ============ phase B: CA back + FFN1 ============
            X1 = self.attn_back(
                X, E, cc_out, NIN, I["wo"], BO, "ca",
                lambda: self.resid.tile([128, CHUNK], F32R, tag="resid", name="resid"))
            X2 = self.ffn(X1, "f1w1", "f1w2", F1B1, F1B2, LNG["ln3"],
                          LNB["ln3"])

            # ============ phase C: SA front ============
            E2 = self.attn_front(X2, I["saq"], SAQ, LNG["ln4"], LNB["ln4"])
            cc2_in = dram.tile([D + 1, C], F32, tag="cc_sa_in", name="cc_sa_in")
            cc2_out = dram.tile([D + 1, C], F32, tag="cc_sa_out", name="cc_sa_out")
            with tc.tile_pool(name="w_kv2", bufs=1) as w_kv2, \
                 tc.tile_pool(name="kvp2", bufs=2) as kvp2, \
                 tc.tile_pool(name="kep2", bufs=2) as kep2, \
                 tc.tile_pool(name="smallp2", bufs=4) as smallp2, \
                 tc.tile_pool(name="ctxsb2", bufs=1) as ctxsbp2, \
                 tc.tile_pool(name="p_ctx2", bufs=1, space="PSUM") as p_ctx2, \
                 tc.tile_pool(name="p_ks2", bufs=1, space="PSUM") as p_ks2:
                SWK = self.load_w512(I["sak"], w_kv2, "sak")
                SWV = self.load_w512(I["sav"], w_kv2, "sav")
                CTXA2 = ctxsbp2.tile([D, C], F32, tag="ctxacc2", name="ctxacc2")
                KSA2 = ctxsbp2.tile([1, C], F32, tag="ksacc2", name="ksacc2")
                for ch in range(NCH):
                    xn = self.layer_norm(X2[ch], LNG["ln4"], LNB["ln4"])
                    ctx_ps = p_ctx2.tile([D, C], F32, tag="ctx2", name="ctx2")
                    ks_ps = p_ks2.tile([1, C], F32, tag="ks2", name="ks2")
                    for t in range(FT):
                        kps = self.proj_tm_psum(SWK, xn, t, SAKR)
                        kn = self.softmax_token_major(kps, kvp2, kep2, smallp2)
                        vps = self.proj_tm_psum(SWV, xn, t, SAVR)
                        vn = kvp2.tile([128, C], F32R, tag="vn", name="vn")
                        nc.scalar.activation(out=vn, in_=vps, func=AF.Copy,
                                             bias=0.0, scale=1.0)
                        for h in range(H):
                            nc.tensor.matmul(
                                ctx_ps[:, D * h : D * (h + 1)],
                                kn[:, D * h : D * (h + 1)],
                                vn[:, D * h : D * (h + 1)],
                                start=(t == 0 and h == 0),
                                stop=(t == FT - 1 and h == H - 1))
                        nc.tensor.matmul(ks_ps, self.ONESR, kn,
                                         start=(t == 0),
                                         stop=(t == FT - 1))
                    if ch == 0:
                        nc.vector.tensor_copy(CTXA2, ctx_ps)
                        nc.vector.tensor_copy(KSA2, ks_ps)
                    else:
                        nc.vector.tensor_tensor(out=CTXA2, in0=CTXA2,
                                                in1=ctx_ps, op=OP.add)
                        nc.vector.tensor_tensor(out=KSA2, in0=KSA2,
                                                in1=ks_ps, op=OP.add)
                nc.sync.dma_start(out=cc2_in[0:D, :], in_=CTXA2)
                nc.sync.dma_start(out=cc2_in[D : D + 1, :], in_=KSA2)
            nc.gpsimd.collective_compute(
                "AllReduce", OP.add, replica_groups=GROUPS,
                ins=[cc2_in[:].opt()], outs=[cc2_out[:].opt()])

            # ============ phase D: SA back + FFN2 ============
            X3 = self.attn_back(
                X2, E2, cc2_out, 1, I["sao"], SAO, "sa",
                lambda: self.resid.tile([128, CHUNK], F32R, tag="resid", name="resid"))
            XF = self.ffn(X3, "f2w1", "f2w2", F2B1, F2B2, LNG["ln5"],
                          LNB["ln5"])

            for ch in range(NCH):
                for m in range(FT):
                    nc.sync.dma_start(
                        out=self.out_t[128 * m : 128 * (m + 1),
                                       CHUNK * ch : CHUNK * (ch + 1)],
                        in_=XF[ch][m].bitcast(F32))


# ---------------------------------------------------------------------------
# host side
# ---------------------------------------------------------------------------
_PROGRAM = None
LAST_RESULTS = None


def _cols(v, nt):
    return np.ascontiguousarray(np.asarray(v, np.float32).reshape(nt, 128).T)


def _host_consts():
    sgbase = np.zeros((FT, 128, 24), np.float32)
    sel8 = np.zeros((FT, 8, 128), np.float32)
    for c in range(FT):
        for p in range(128):
            h = 2 * c + (1 if p >= 64 else 0)
            sgbase[c, p, h] = 1.0
            sel8[c, h, p] = 1.0
    return {
        "ones_c": np.ones((1, 128), np.float32),
        "ones_r": np.ones((128, 1), np.float32),
        "sgbase": sgbase,
        "sel8": sel8,
        "zz": np.zeros((128, 128), np.float32),
    }


def _make_in_maps(inputs):
    f = lambda k: np.asarray(inputs[k], np.float32)
    shared = {
        "wq": np.ascontiguousarray(f("ca_wq").T),
        "wo": np.ascontiguousarray(f("ca_wo").T),
        "saq": np.ascontiguousarray(f("sa_wq").T),
        "sak": np.ascontiguousarray(f("sa_wk").T),
        "sav": np.ascontiguousarray(f("sa_wv").T),
        "sao": np.ascontiguousarray(f("sa_wo").T),
        "wk": np.ascontiguousarray(f("ca_wk").transpose(0, 2, 1)),
        "wv": np.ascontiguousarray(f("ca_wv").transpose(0, 2, 1)),
        "f1w1": np.ascontiguousarray(f("ffn1_w1").T),
        "f1w2": np.ascontiguousarray(f("ffn1_w2").T),
        "f2w1": np.ascontiguousarray(f("ffn2_w1").T),
        "f2w2": np.ascontiguousarray(f("ffn2_w2").T),
        "bq_c": _cols(f("ca_bq"), FT),
        "bo_c": _cols(f("ca_bo"), FT),
        "saq_c": _cols(f("sa_bq"), FT),
        "sao_c": _cols(f("sa_bo"), FT),
        "f1b1_c": _cols(f("ffn1_b1"), IT),
        "f1b2_c": _cols(f("ffn1_b2"), FT),
        "f2b1_c": _cols(f("ffn2_b1"), IT),
        "f2b2_c": _cols(f("ffn2_b2"), FT),
        "bk_r": np.ascontiguousarray(f("ca_bk").reshape(NIN, 1, C)),
        "bv_r": np.ascontiguousarray(f("ca_bv").reshape(NIN, 1, C)),
        "sak_r": np.ascontiguousarray(f("sa_bk").reshape(1, C)),
        "sav_r": np.ascontiguousarray(f("sa_bv").reshape(1, C)),
        "ln1_g": _cols(f("ln1_g"), FT), "ln1_b": _cols(f("ln1_b"), FT),
        "ln3_g": _cols(f("ln3_g"), FT), "ln3_b": _cols(f("ln3_b"), FT),
        "ln4_g": _cols(f("ln4_g"), FT), "ln4_b": _cols(f("ln4_b"), FT),
        "ln5_g": _cols(f("ln5_g"), FT), "ln5_b": _cols(f("ln5_b"), FT),
        "ln2_g": np.stack([_cols(f("ln2_g")[i], FT) for i in range(NIN)]),
        "ln2_b": np.stack([_cols(f("ln2_b")[i], FT) for i in range(NIN)]),
    }
    shared.update(_host_consts())

    x = f("x")
    ys = f("ys")
    in_maps = []
    for core in range(N_CORES):
        b, half = core // 2, core % 2
        lo, hi = half * NTOK, (half + 1) * NTOK
        m = dict(shared)
        m["xT"] = np.ascontiguousarray(x[b, lo:hi, :].T)
        m["ysT"] = np.ascontiguousarray(ys[:, b, lo:hi, :].transpose(0, 2, 1))
        in_maps.append(m)
    return in_maps


def kernel(**inputs):
    global _PROGRAM, LAST_RESULTS
    if _PROGRAM is None:
        _PROGRAM = _build_program()
    nc = _PROGRAM
    in_maps = _make_in_maps(inputs)

    trace = os.environ.get("BASS_TRACE", "") not in ("", "0")
    res = run_bass_kernel_spmd(nc, in_maps, core_ids=list(range(N_CORES)),
                               trace=trace)
    LAST_RESULTS = res

    out = np.empty((B, T, C), np.float32)
    for core in range(N_CORES):
        b, half = core // 2, core % 2
        lo, hi = half * NTOK, (half + 1) * NTOK
        out[b, lo:hi, :] = res.results[core]["outT"].T
    return out



# revision 19
# speedup vs baseline: 1.4159x; 1.4159x over previous
"""Trainium2 Bass kernel for nn_CrossAttentionBlock (B=4, T=4096, C=512, H=8,
INNER=2048, NIN=2) on 8 NeuronCores.

Sharding: core c handles batch b=c//2, token half h=c%2 (2048 tokens each).
Cross-core coupling: only the linear-attention context (ctx = k^T v) and
k_sum, pair-wise AllReduced (cores 2b, 2b+1).

v2 design notes (vs the v1 feature-major baseline):
- LN gains folded into weights host-side; LN bias folded into projection
  biases. The per-token mean-shift enters each projection through a K<=2
  "seed" matmul (beta (x) u + 1 (x) b') that replaces the plain bias matmul.
- inv_std = exp(-0.5*ln(var+eps)) so LN, softmax-exp and all copies share
  ONE scalar-engine activation table (natural_log_exp family); gelu is the
  only other table -> ~4 table loads total.
- All heavy matmuls in bf16 (1 cyc/row incl. the 65-row ctx matmuls that
  were 4 cyc/row in fp32r); LN stats matmuls stay fp32r; residual fp32.
- scalar_tensor_tensor reads broadcast/projection results straight from
  PSUM (no PSUM->SBUF copy ops); squares and k-normalization run on the
  idle gpsimd (Pool) engine; reciprocals use reciprocal_approx_fast.
- attn combine: out = E/S + sum_i (E@ctx_i)/G_i is computed as
  qn + sum_i BD_i.T @ (E o bc(1/G_i)) with the per-head scale applied to E
  BEFORE the block-diag matmul (legal: scale is constant within a head),
  so the BD products accumulate in PSUM.
- ks_i emerges as an extra all-ones column in the ctx matmul, pre-laid-out
  in the [128, 260] folded DRAM tile that the AllReduce moves, so SG/BD
  assembly after the collective is a handful of plain DMAs.
- Phase order: kv/ctx first, then the collective overlaps the q/E (and E2)
  production.
"""
import os
import numpy as np

import concourse.bass as bass
import concourse.tile as tile
from concourse import mybir
from concourse.vector_clock import ScopedClock
from concourse.bass_utils import run_bass_kernel_spmd

F32 = mybir.dt.float32
F32R = mybir.dt.float32r
BF16 = mybir.dt.bfloat16
AF = mybir.ActivationFunctionType
OP = mybir.AluOpType

B, T, C, H, D, INNER, NIN = 4, 4096, 512, 8, 64, 2048, 2
N_CORES = 8
NTOK = 2048          # tokens per core
CHUNK = 512          # tokens per chunk
NCH = NTOK // CHUNK  # 4 chunks
FT = C // 128        # 4 feature tiles
IT = INNER // 128    # 16 inner tiles
HB = 65              # head block width in ctx psum (64 v cols + 1 ks col)
LN_EPS = 1e-5
GROUPS = [[0, 1], [2, 3], [4, 5], [6, 7]]

_split_counter = [0]


def _split_multi_waits(nc):
    """This walrus build only supports one sync-wait per instruction; move
    extra waits onto same-engine NoOps placed immediately before."""
    for f in nc.m.functions:
        for blk in f.blocks:
            out = []
            changed = False
            for inst in blk.instructions:
                si = inst.sync_info
                if si is not None and si.on_wait and len(si.on_wait) > 1:
                    waits = list(si.on_wait)
                    for w in waits[:-1]:
                        _split_counter[0] += 1
                        nop = mybir.InstNoOp(
                            name=f"I-waitsplit-{_split_counter[0]}", ins=[], outs=[]
                        )
                        nop.engine = inst.engine
                        nop.sync_info = mybir.SyncInfo(on_wait=[w], on_update=[])
                        out.append(nop)
                    si.on_wait = waits[-1:]
                    inst.sync_info = si
                    changed = True
                out.append(inst)
            if changed:
                blk.instructions = out


class _TC(tile.TileContext):
    def _drain_and_barrier(self, tick_clock, wait_clock):
        drain_inst = self.nc.sync.drain()
        wait_clock.add_sem_waits(
            drain_inst.ins, ScopedClock({None: tick_clock.global_clock})
        )
        si = drain_inst.ins.sync_info
        if si is not None and si.on_wait and len(si.on_wait) > 1:
            waits = list(si.on_wait)
            si.on_wait = waits[:1]
            drain_inst.ins.sync_info = si
            for i in range(1, len(waits)):
                extra = self.nc.sync.drain()
                esi = extra.ins.sync_info
                if esi is None:
                    extra.ins.sync_info = mybir.SyncInfo(
                        on_wait=waits[i : i + 1], on_update=[]
                    )
                else:
                    esi.on_wait = waits[i : i + 1]
                    extra.ins.sync_info = esi
        self.nc.all_engine_barrier()
        assert self.sems is not None
        popped = self.nc._tile_sem_poison_stack.pop()
        assert popped is self._sem_poison
        self.nc.clear_and_free_semaphores(list(self.sems.allocated().values()))
        self.nc.all_engine_barrier()


def _build_program(split=True):
    nc = bass.Bass("TRN2", target_bir_lowering=False, debug=False, num_devices=N_CORES)
    I = {}

    def di(name, shape, dt=F32):
        I[name] = nc.dram_tensor(name, list(shape), dt, kind="ExternalInput").ap()

    di("xT", [C, NTOK])
    di("ysT", [NIN, C, NTOK])
    # bf16 weights, [in, out] layout, LN gains folded where applicable
    di("wq", [C, C], BF16)
    di("wo", [C, C], BF16)
    di("saq", [C, C], BF16)
    di("sao", [C, C], BF16)
    di("sak", [C, C], BF16)
    di("sav", [C, C], BF16)
    di("wk", [NIN, C, C], BF16)
    di("wv", [NIN, C, C], BF16)
    di("f1w1", [C, INNER], BF16)
    di("f1w2", [INNER, C], BF16)
    di("f2w1", [C, INNER], BF16)
    di("f2w2", [INNER, C], BF16)
    # seeds: FM u-rows [FT, 1, 128]; TM [2, C] = [u; b'] stacks
    di("fmsq", [FT, 1, 128], BF16)
    di("fmssaq", [FT, 1, 128], BF16)
    di("kvsd", [NIN, 2, 2, C], BF16)   # [i][k/v] -> [u; b'] rows
    di("sasd", [2, 2, C], BF16)        # [k/v] -> [u; b'] rows
    # bias cols fp32 (per-partition activation biases / stt scalars)
    di("bq_c", [128, FT])
    di("bsaq_c", [128, FT])
    di("bo_c", [128, FT])
    di("bsao_c", [128, FT])
    di("f1b1_c", [128, IT])
    di("f2b1_c", [128, IT])
    di("f1b2_c", [128, FT])
    di("f2b2_c", [128, FT])
    di("ones_c", [1, 128])
    di("ones_r", [128, 1])
    di("sgbase", [FT, 128, 24])
    di("sel8", [FT, 8, 128], BF16)

    out_t = nc.dram_tensor("outT", [C, NTOK], F32, kind="ExternalOutput").ap()

    with _TC(nc) as tc:
        _Emitter(nc, tc, I, out_t).run()
    if split:
        _split_multi_waits(nc)
    from concourse.library_overlay import lower_extended_insts
    lower_extended_insts(nc)
    return nc


class _Emitter:
    def __init__(self, nc, tc, I, out_t):
        self.nc, self.tc, self.I, self.out_t = nc, tc, I, out_t

    # ---------------- layer norm front ----------------
    def ln_front(self, x_tiles, sq_engine="pool"):
        """Stats + rows for LN on fp32(r) feature-major tiles.
        Returns (A_ps [128,CHUNK] f32 PSUM broadcast of inv_std,
                 betaones [2,CHUNK] bf16 SBUF: row0=-m*inv_std, row1=1)."""
        nc = self.nc
        s_ps = self.p_stats.tile([1, CHUNK], F32, tag="stats", name="stats")
        for k in range(FT):
            nc.tensor.matmul(s_ps, self.ONESR, x_tiles[k],
                             start=(k == 0), stop=(k == FT - 1))
        xsq = []
        for k in range(FT):
            sq = self.sqp.tile([128, CHUNK], F32R, tag="xsq", name="xsq")
            if sq_engine == "pool":
                with nc.allow_low_precision(reason="fp32r feeds matmul"):
                    nc.gpsimd.tensor_tensor(out=sq, in0=x_tiles[k].bitcast(F32),
                                            in1=x_tiles[k].bitcast(F32),
                                            op=OP.mult)
            else:
                nc.scalar.activation(out=sq, in_=x_tiles[k].bitcast(F32),
                                     func=AF.Square)
            xsq.append(sq)
        q_ps = self.p_stats.tile([1, CHUNK], F32, tag="stats", name="stats")
        for k in range(FT):
            nc.tensor.matmul(q_ps, self.ONESR, xsq[k],
                             start=(k == 0), stop=(k == FT - 1))
        # rows: mrow = -mean; var = E[x^2] - mean^2; alpha = rsqrt(var+eps)
        mrow = self.rows.tile([1, CHUNK], F32, tag="rows", name="rows")
        nc.vector.tensor_scalar(out=mrow, in0=s_ps, scalar1=-1.0 / C,
                                scalar2=None, op0=OP.mult)
        m2 = self.rows.tile([1, CHUNK], F32, tag="rows", name="rows")
        nc.vector.tensor_tensor(out=m2, in0=mrow, in1=mrow, op=OP.mult)
        var = self.rows.tile([1, CHUNK], F32, tag="rows", name="rows")
        nc.vector.scalar_tensor_tensor(out=var, in0=q_ps, scalar=1.0 / C,
                                       in1=m2, op0=OP.mult, op1=OP.subtract)
        lnv = self.rows.tile([1, CHUNK], F32, tag="rows", name="rows")
        nc.scalar.activation(out=lnv, in_=var, func=AF.Ln, bias=self.EPS,
                             scale=1.0)
        alpha = self.rows.tile([1, CHUNK], F32R, tag="rows", name="rows")
        with nc.allow_low_precision(reason="fp32r feeds matmul"):
            nc.scalar.activation(out=alpha, in_=lnv, func=AF.Exp,
                                 bias=self.ZERO1, scale=-0.5)
        bo = self.bop.tile([2, CHUNK], BF16, tag="bo", name="bo")
        nc.vector.memset(bo, 1.0)
        with nc.allow_low_precision(reason="seed row"):
            nc.vector.tensor_tensor(out=bo[0:1, :], in0=mrow,
                                    in1=alpha.bitcast(F32), op=OP.mult)
        a_ps = self.p_bc.tile([128, CHUNK], F32, tag="bc", name="bc")
        nc.tensor.matmul(a_ps, self.ONESC, alpha, start=True, stop=True)
        return a_ps, bo

    def ln_apply(self, x_tiles, a_ps, pool, tag):
        """xn[k] = x[k] * bc(inv_std)  (bf16, mean-shift via seed matmuls)"""
        nc = self.nc
        outs = []
        for k in range(FT):
            xk = pool.tile([128, CHUNK], BF16, tag=tag, name=tag)
            nc.vector.scalar_tensor_tensor(
                out=xk, in0=x_tiles[k].bitcast(F32), scalar=1.0, in1=a_ps,
                op0=OP.mult, op1=OP.mult)
            outs.append(xk)
        return outs

    def load_w512(self, ap, pool, tag, width=C):
        tiles = []
        for k in range(FT):
            t = pool.tile([128, width], BF16, tag=f"{tag}{k}", name=f"{tag}{k}")
            self.nc.sync.dma_start(out=t, in_=ap[128 * k : 128 * (k + 1), :])
            tiles.append(t)
        return tiles

    # ---------------- kv + ctx pipeline (token-major) ----------------
    def kv_ctx(self, zy, bo, WK, WV, sdk, sdv, ctx_ps, first, last):
        """One (chunk, input) step: k/v proj + softmax-k + ctx accumulation.
        zy: 4 bf16 FM tiles; bo: [2,CHUNK] betaones; sdk/sdv: [2,C] moving
        seed rows; ctx_ps: [128, 4*HB] psum tile (even heads rows 0:64,
        odd heads rows 64:128)."""
        nc = self.nc
        for t in range(FT):
            kps = self.p_kv.tile([128, C], F32, tag="kv", name="kv")
            nc.tensor.matmul(kps, bo[:, 128 * t : 128 * (t + 1)], sdk,
                             start=True, stop=False)
            for k in range(FT):
                nc.tensor.matmul(kps, zy[k][:, 128 * t : 128 * (t + 1)],
                                 WK[k], start=False, stop=(k == FT - 1))
            kE = self.kep.tile([128, C], BF16, tag="kE", name="kE")
            nc.scalar.activation(out=kE, in_=kps, func=AF.Exp,
                                 bias=self.ZERO128, scale=1.0)
            ssum = self.smallp.tile([128, H], F32, tag="ssum", name="ssum")
            nc.vector.tensor_reduce(
                out=ssum, in_=kE.rearrange("p (h d) -> p h d", d=D),
                axis=mybir.AxisListType.X, op=OP.add)
            rsum = self.smallp.tile([128, H], F32, tag="rsum", name="rsum")
            nc.vector.reciprocal(out=rsum, in_=ssum)
            kn = self.knp.tile([128, C], BF16, tag="kn", name="kn")
            for h in range(H):
                nc.vector.tensor_scalar(
                    out=kn[:, D * h : D * (h + 1)],
                    in0=kE[:, D * h : D * (h + 1)],
                    scalar1=rsum[:, h : h + 1], scalar2=None, op0=OP.mult)
            vps = self.p_kv.tile([128, C], F32, tag="kv", name="kv")
            nc.tensor.matmul(vps, bo[:, 128 * t : 128 * (t + 1)], sdv,
                             start=True, stop=False)
            for k in range(FT):
                nc.tensor.matmul(vps, zy[k][:, 128 * t : 128 * (t + 1)],
                                 WV[k], start=False, stop=(k == FT - 1))
            va = self.vap.tile([128, H * HB], BF16, tag="va", name="va")
            nc.scalar.activation(
                out=va.rearrange("p (h b) -> p h b", b=HB)[:, :, 0:D],
                in_=vps.rearrange("p (h d) -> p h d", d=D),
                func=AF.Copy, bias=0.0, scale=1.0)
            nc.vector.memset(
                va.rearrange("p (h b) -> p h b", b=HB)[:, :, D : D + 1], 1.0)
            st = first and t == 0
            sp = last and t == FT - 1
            for h in range(H):
                half, c = h % 2, h // 2
                nc.tensor.matmul(
                    ctx_ps[64 * half : 64 * half + 64, HB * c : HB * (c + 1)],
                    kn[:, D * h : D * (h + 1)],
                    va[:, HB * h : HB * (h + 1)],
                    start=st, stop=sp,
                    tile_position=(0, 64 * half))

    # ---------------- attention back ----------------
    def attn_back(self, X, E, cc_out, n_in, wo_ap, bo_cols, tagp, Xnew_pool):
        """out = E/S + sum_i BD_i.T @ (E o bc(1/G_i)); then wo proj+residual."""
        nc, tc, I = self.nc, self.tc, self.I
        Xout = [[None] * FT for _ in range(NCH)]
        cc = (lambda i: cc_out[i]) if n_in > 1 else (lambda i: cc_out)
        ncols = 8 + 8 * n_in
        with tc.tile_pool(name=f"w_{tagp}", bufs=1) as w_o, \
             tc.tile_pool(name=f"as_{tagp}", bufs=1) as attn_s, \
             tc.tile_pool(name=f"ao_{tagp}", bufs=10) as aop, \
             tc.tile_pool(name=f"rec_{tagp}", bufs=4) as recp, \
             tc.tile_pool(name=f"pg_{tagp}", bufs=1, space="PSUM") as p_g, \
             tc.tile_pool(name=f"pr_{tagp}", bufs=3, space="PSUM") as p_r:
            WO = self.load_w512(wo_ap, w_o, "wo")
            # SG tiles: base pattern + ks columns from cc_out, then -> bf16
            SGT = []
            for c in range(FT):
                sgf = attn_s.tile([128, ncols], F32, tag=f"sgf{c}", name=f"sgf{c}")
                nc.sync.dma_start(out=sgf, in_=I["sgbase"][c][:, 0:ncols])
                for i in range(n_in):
                    col = 8 + 8 * i + 2 * c
                    nc.gpsimd.dma_start(
                        out=sgf[0:64, col : col + 1],
                        in_=cc(i)[0:64, HB * c + D : HB * c + D + 1])
                    nc.gpsimd.dma_start(
                        out=sgf[64:128, col + 1 : col + 2],
                        in_=cc(i)[64:128, HB * c + D : HB * c + D + 1])
                sg = attn_s.tile([128, ncols], BF16, tag=f"sg{c}", name=f"sg{c}")
                nc.scalar.activation(out=sg, in_=sgf, func=AF.Copy, bias=0.0,
                                     scale=1.0)
                SGT.append(sg)
            BD = [[None] * FT for _ in range(n_in)]
            for i in range(n_in):
                for c in range(FT):
                    bdf = attn_s.tile([128, 128], F32, tag=f"bdf{i}_{c}",
                                      name=f"bdf{i}_{c}")
                    nc.vector.memset(bdf, 0.0)
                    nc.gpsimd.dma_start(
                        out=bdf[0:64, 0:64],
                        in_=cc(i)[0:64, HB * c : HB * c + D])
                    nc.gpsimd.dma_start(
                        out=bdf[64:128, 64:128],
                        in_=cc(i)[64:128, HB * c : HB * c + D])
                    bd = attn_s.tile([128, 128], BF16, tag=f"bd{i}_{c}",
                                     name=f"bd{i}_{c}")
                    nc.scalar.activation(out=bd, in_=bdf, func=AF.Copy,
                                         bias=0.0, scale=1.0)
                    BD[i][c] = bd

            for ch in range(NCH):
                recs = []
                for j in range(1 + n_in):
                    gps = p_g.tile([8, CHUNK], F32, tag="gps", name="gps")
                    for c in range(FT):
                        nc.tensor.matmul(gps, SGT[c][:, 8 * j : 8 * (j + 1)],
                                         E[ch][c], start=(c == 0),
                                         stop=(c == FT - 1))
                    r = recp.tile([8, CHUNK], F32, tag="rec", name="rec")
                    nc.vector.reciprocal_approx_fast(out=r, in_=gps)
                    rb = recp.tile([8, CHUNK], BF16, tag="recb", name="recb")
                    nc.gpsimd.tensor_scalar(out=rb, in0=r, scalar1=1.0,
                                            scalar2=None, op0=OP.mult)
                    recs.append(rb)
                outc = []
                for c in range(FT):
                    Rps = []
                    for j in range(1 + n_in):
                        rp = p_r.tile([128, CHUNK], F32, tag="R", name="R")
                        nc.tensor.matmul(rp, self.SEL8[c], recs[j],
                                         start=True, stop=True)
                        Rps.append(rp)
                    qn = aop.tile([128, CHUNK], BF16, tag="qn", name="qn")
                    nc.vector.scalar_tensor_tensor(
                        out=qn, in0=E[ch][c], scalar=1.0, in1=Rps[0],
                        op0=OP.mult, op1=OP.mult)
                    bd_ps = self.p_mm.tile([128, CHUNK], F32, tag="mm", name="mm")
                    for i in range(n_in):
                        qh = aop.tile([128, CHUNK], BF16, tag="qh", name="qh")
                        nc.vector.scalar_tensor_tensor(
                            out=qh, in0=E[ch][c], scalar=1.0, in1=Rps[1 + i],
                            op0=OP.mult, op1=OP.mult)
                        nc.tensor.matmul(bd_ps, BD[i][c], qh,
                                         start=(i == 0), stop=(i == n_in - 1))
                    ao = aop.tile([128, CHUNK], BF16, tag="ao", name="ao")
                    nc.vector.scalar_tensor_tensor(
                        out=ao, in0=qn, scalar=0.0, in1=bd_ps,
                        op0=OP.add, op1=OP.add)
                    outc.append(ao)
                for m in range(FT):
                    wps = self.p_mm.tile([128, CHUNK], F32, tag="mm", name="mm")
                    for c in range(FT):
                        nc.tensor.matmul(wps, WO[c][:, 128 * m : 128 * (m + 1)],
                                         outc[c], start=(c == 0),
                                         stop=(c == FT - 1))
                    xo = Xnew_pool.tile([128, CHUNK], F32R, tag="resid",
                                        name="resid")
                    with nc.allow_low_precision(reason="fp32r resid"):
                        nc.vector.scalar_tensor_tensor(
                            out=xo, in0=X[ch][m].bitcast(F32),
                            scalar=bo_cols[:, m : m + 1], in1=wps,
                            op0=OP.add, op1=OP.add)
                    Xout[ch][m] = xo
        return Xout

    # ---------------- FFN ----------------
    def ffn(self, Xin, w1name, w2name, B1, B2):
        nc, tc, I = self.nc, self.tc, self.I
        Xout = [[None] * FT for _ in range(NCH)]
        with tc.tile_pool(name=w1name, bufs=1) as w1p, \
             tc.tile_pool(name=w2name + "s", bufs=8) as w2p, \
             tc.tile_pool(name=w1name + "h", bufs=4) as hp, \
             tc.tile_pool(name=w1name + "x", bufs=10) as xnp, \
             tc.tile_pool(name=w1name + "b", bufs=2) as bbp, \
             tc.tile_pool(name=w1name + "p", bufs=4, space="PSUM") as p_ffn:
            W1 = []
            for k in range(FT):
                t = w1p.tile([128, INNER], BF16, tag=f"w1_{k}", name=f"w1_{k}")
                nc.sync.dma_start(
                    out=t, in_=I[w1name][128 * k : 128 * (k + 1), :])
                W1.append(t)
            for ch in range(NCH):
                a_ps, bo = self.ln_front(Xin[ch])
                b_ps = self.p_bc.tile([128, CHUNK], F32, tag="bc", name="bc")
                nc.tensor.matmul(b_ps, self.ONESCB, bo[0:1, :], start=True,
                                 stop=True)
                bsb = bbp.tile([128, CHUNK], F32, tag="bsb", name="bsb")
                nc.scalar.activation(out=bsb, in_=b_ps, func=AF.Copy,
                                     bias=0.0, scale=1.0)
                xn = []
                for k in range(FT):
                    u = xnp.tile([128, CHUNK], F32, tag="u", name="u")
                    nc.vector.scalar_tensor_tensor(
                        out=u, in0=Xin[ch][k].bitcast(F32), scalar=1.0,
                        in1=a_ps, op0=OP.mult, op1=OP.mult)
                    xk = xnp.tile([128, CHUNK], BF16, tag="xn", name="xn")
                    nc.gpsimd.tensor_tensor(out=xk, in0=u, in1=bsb, op=OP.add)
                    xn.append(xk)
                ops = [p_ffn.tile([128, CHUNK], F32, tag="ffn", name="ffn")
                       for _ in range(FT)]
                for k in range(IT):
                    hps = self.p_mm.tile([128, CHUNK], F32, tag="mm", name="mm")
                    for c in range(FT):
                        nc.tensor.matmul(hps, W1[c][:, 128 * k : 128 * (k + 1)],
                                         xn[c], start=(c == 0),
                                         stop=(c == FT - 1))
                    h = hp.tile([128, CHUNK], BF16, tag="h", name="h")
                    nc.scalar.activation(out=h, in_=hps, func=AF.Gelu_apprx_tanh,
                                         bias=B1[:, k : k + 1], scale=1.0)
                    w2t = w2p.tile([128, C], BF16, tag="w2s", name="w2s")
                    nc.sync.dma_start(
                        out=w2t, in_=I[w2name][128 * k : 128 * (k + 1), :])
                    for m in range(FT):
                        nc.tensor.matmul(ops[m],
                                         w2t[:, 128 * m : 128 * (m + 1)], h,
                                         start=(k == 0), stop=(k == IT - 1))
                for m in range(FT):
                    xo = self.resid.tile([128, CHUNK], F32R, tag="resid",
                                         name="resid")
                    with nc.allow_low_precision(reason="fp32r resid"):
                        nc.vector.scalar_tensor_tensor(
                            out=xo, in0=Xin[ch][m].bitcast(F32),
                            scalar=B2[:, m : m + 1], in1=ops[m],
                            op0=OP.add, op1=OP.add)
                    Xout[ch][m] = xo
        return Xout

    # ---------------- q / E production (feature-major) ----------------
    def q_exp(self, Xin, wname, fmname, bcol, lnpack, Epool, sq_engine="pool"):
        """E[ch][m] = exp(Wq_g @ (x o bc(alpha)) + u (x) beta + b') for all
        chunks. lnpack: None (LN computed here per chunk) or a list of
        (bo, xn_tiles) per chunk."""
        nc, tc, I = self.nc, self.tc, self.I
        E = [[None] * FT for _ in range(NCH)]
        with tc.tile_pool(name=f"w_{wname}", bufs=1) as w_q, \
             tc.tile_pool(name=f"fms_{wname}", bufs=1) as fmsp, \
             tc.tile_pool(name=f"xn_{wname}", bufs=8) as xnp:
            WQ = self.load_w512(I[wname], w_q, "wq")
            FMS = []
            for m in range(FT):
                s = fmsp.tile([1, 128], BF16, tag=f"fms{m}", name=f"fms{m}")
                nc.sync.dma_start(out=s, in_=I[fmname][m])
                FMS.append(s)
            for ch in range(NCH):
                if lnpack is None:
                    a_ps, bo = self.ln_front(Xin[ch], sq_engine=sq_engine)
                    xn = self.ln_apply(Xin[ch], a_ps, xnp, "xn")
                else:
                    bo, xn = lnpack[ch]
                for m in range(FT):
                    ps = self.p_mm.tile([128, CHUNK], F32, tag="mm", name="mm")
                    nc.tensor.matmul(ps, FMS[m], bo[0:1, :], start=True,
                                     stop=False)
                    for k in range(FT):
                        nc.tensor.matmul(ps, WQ[k][:, 128 * m : 128 * (m + 1)],
                                         xn[k], start=False,
                                         stop=(k == FT - 1))
                    e = Epool.tile([128, CHUNK], BF16, tag="E", name="E")
                    nc.scalar.activation(out=e, in_=ps, func=AF.Exp,
                                         bias=bcol[:, m : m + 1], scale=1.0)
                    E[ch][m] = e
        return E

    # ---------------- main ----------------
    def run(self):
        nc, tc, I = self.nc, self.tc, self.I
        from contextlib import ExitStack

        with ExitStack() as ctx:
            const = ctx.enter_context(tc.tile_pool(name="const", bufs=1))
            self.resid = ctx.enter_context(tc.tile_pool(name="resid", bufs=20))
            self.epool = ctx.enter_context(tc.tile_pool(name="E", bufs=16))
            self.xn4p = ctx.enter_context(tc.tile_pool(name="xn4", bufs=16))
            self.rows = ctx.enter_context(tc.tile_pool(name="rows", bufs=10))
            self.bop = ctx.enter_context(tc.tile_pool(name="bop", bufs=10))
            self.sqp = ctx.enter_context(tc.tile_pool(name="sqp", bufs=5))
            dram = ctx.enter_context(tc.tile_pool(name="dram", bufs=1,
                                                  space="DRAM"))
            self.p_mm = ctx.enter_context(
                tc.tile_pool(name="p_mm", bufs=2, space="PSUM"))
            self.p_stats = ctx.enter_context(
                tc.tile_pool(name="p_stats", bufs=1, space="PSUM"))
            self.p_bc = ctx.enter_context(
                tc.tile_pool(name="p_bc", bufs=1, space="PSUM"))

            # ---------------- constants ----------------
            self.EPS = const.tile([1, 1], F32, tag="eps", name="eps")
            nc.vector.memset(self.EPS, LN_EPS)
            self.ZERO1 = const.tile([1, 1], F32, tag="z1", name="z1")
            nc.vector.memset(self.ZERO1, 0.0)
            self.ZERO128 = const.tile([128, 1], F32, tag="z128", name="z128")
            nc.vector.memset(self.ZERO128, 0.0)
            self.ONESC = const.tile([1, 128], F32R, tag="onesc", name="onesc")
            nc.sync.dma_start(out=self.ONESC, in_=I["ones_c"].bitcast(F32R))
            self.ONESCB = const.tile([1, 128], BF16, tag="onescb", name="onescb")
            nc.vector.memset(self.ONESCB, 1.0)
            self.ONESR = const.tile([128, 1], F32R, tag="onesr", name="onesr")
            nc.sync.dma_start(out=self.ONESR, in_=I["ones_r"].bitcast(F32R))
            self.SEL8 = []
            for c in range(FT):
                s = const.tile([8, 128], BF16, tag=f"sel8_{c}", name=f"sel8_{c}")
                nc.sync.dma_start(out=s, in_=I["sel8"][c])
                self.SEL8.append(s)

            def cols_tile(name, nt):
                t = const.tile([128, nt], F32, tag=name)
                nc.sync.dma_start(out=t, in_=I[name])
                return t

            BQ = cols_tile("bq_c", FT)
            BSAQ = cols_tile("bsaq_c", FT)
            BO = cols_tile("bo_c", FT)
            BSAO = cols_tile("bsao_c", FT)
            F1B1 = cols_tile("f1b1_c", IT)
            F1B2 = cols_tile("f1b2_c", FT)
            F2B1 = cols_tile("f2b1_c", IT)
            F2B2 = cols_tile("f2b2_c", FT)

            KVSD = []
            for i in range(NIN):
                sdk = const.tile([2, C], BF16, tag=f"sdk{i}", name=f"sdk{i}")
                nc.sync.dma_start(out=sdk, in_=I["kvsd"][i, 0])
                sdv = const.tile([2, C], BF16, tag=f"sdv{i}", name=f"sdv{i}")
                nc.sync.dma_start(out=sdv, in_=I["kvsd"][i, 1])
                KVSD.append((sdk, sdv))
            SASDK = const.tile([2, C], BF16, tag="sasdk", name="sasdk")
            nc.sync.dma_start(out=SASDK, in_=I["sasd"][0])
            SASDV = const.tile([2, C], BF16, tag="sasdv", name="sasdv")
            nc.sync.dma_start(out=SASDV, in_=I["sasd"][1])

            # ---------------- residual load ----------------
            X = [[self.resid.tile([128, CHUNK], F32R, tag="resid", name="resid")
                  for _ in range(FT)] for _ in range(NCH)]
            for ch in range(NCH):
                for c in range(FT):
                    nc.sync.dma_start(
                        out=X[ch][c],
                        in_=I["xT"][128 * c : 128 * (c + 1),
                                    CHUNK * ch : CHUNK * (ch + 1)].bitcast(F32R))

            # ============ phase A: CA kv + ctx ============
            cc_in = dram.tile([NIN, 128, FT * HB], F32, tag="cc_ca_in",
                              name="cc_ca_in")
            cc_out = dram.tile([NIN, 128, FT * HB], F32, tag="cc_ca_out",
                               name="cc_ca_out")
            with tc.tile_pool(name="w_kv", bufs=1) as w_kv, \
                 tc.tile_pool(name="ysp", bufs=12) as ysp, \
                 tc.tile_pool(name="zyp", bufs=8) as zyp, \
                 tc.tile_pool(name="kep", bufs=2) as kep, \
                 tc.tile_pool(name="knp", bufs=2) as knp, \
                 tc.tile_pool(name="vap", bufs=2) as vap, \
                 tc.tile_pool(name="smallp", bufs=4) as smallp, \
                 tc.tile_pool(name="ctxsb", bufs=2) as ctxsb, \
                 tc.tile_pool(name="p_ctx", bufs=1, space="PSUM") as p_ctx, \
                 tc.tile_pool(name="p_kv", bufs=2, space="PSUM") as p_kv:
                self.p_kv, self.kep, self.knp = p_kv, kep, knp
                self.vap, self.smallp = vap, smallp
                WK = [self.load_w512(I["wk"][i], w_kv, f"wk{i}")
                      for i in range(NIN)]
                WV = [self.load_w512(I["wv"][i], w_kv, f"wv{i}")
                      for i in range(NIN)]
                CTX = [p_ctx.tile([128, FT * HB], F32, tag=f"ctx{i}",
                                  name=f"ctx{i}") for i in range(NIN)]
                for ch in range(NCH):
                    yt = {}
                    for i in range(NIN):
                        yt[i] = []
                        for c in range(FT):
                            y = ysp.tile([128, CHUNK], F32R, tag="ys", name="ys")
                            nc.sync.dma_start(
                                out=y,
                                in_=I["ysT"][i, 128 * c : 128 * (c + 1),
                                             CHUNK * ch : CHUNK * (ch + 1)
                                             ].bitcast(F32R))
                            yt[i].append(y)
                    for i in range(NIN):
                        a_ps, bo = self.ln_front(yt[i])
                        zy = self.ln_apply(yt[i], a_ps, zyp, "zy")
                        self.kv_ctx(zy, bo, WK[i], WV[i], KVSD[i][0],
                                    KVSD[i][1], CTX[i],
                                    first=(ch == 0), last=(ch == NCH - 1))
                for i in range(NIN):
                    cs = ctxsb.tile([128, FT * HB], F32, tag="ctxsb",
                                    name="ctxsb")
                    nc.scalar.activation(out=cs, in_=CTX[i], func=AF.Copy,
                                         bias=0.0, scale=1.0)
                    nc.sync.dma_start(out=cc_in[i], in_=cs)
            nc.gpsimd.collective_compute(
                "AllReduce", OP.add, replica_groups=GROUPS,
                ins=[cc_in[:].opt()], outs=[cc_out[:].opt()])

            # ============ phase A2: q/E (overlaps AllReduce) ============
            E = self.q_exp(X, "wq", "fmsq", BQ, None, self.epool,
                           sq_engine="scalar")

            # ============ phase B: CA back + FFN1 ============
            X1 = self.attn_back(X, E, cc_out, NIN, I["wo"], BO, "ca",
                                self.resid)
            X2 = self.ffn(X1, "f1w1", "f1w2", F1B1, F1B2)

            # ============ phase C: SA kv + ctx ============
            cc2_in = dram.tile([128, FT * HB], F32, tag="cc_sa_in",
                               name="cc_sa_in")
            cc2_out = dram.tile([128, FT * HB], F32, tag="cc_sa_out",
                                name="cc_sa_out")
            lnpack4 = []
            with tc.tile_pool(name="w_kv2", bufs=1) as w_kv2, \
                 tc.tile_pool(name="kep2", bufs=2) as kep2, \
                 tc.tile_pool(name="knp2", bufs=2) as knp2, \
                 tc.tile_pool(name="vap2", bufs=2) as vap2, \
                 tc.tile_pool(name="smallp2", bufs=4) as smallp2, \
                 tc.tile_pool(name="ctxsb2", bufs=1) as ctxsb2, \
                 tc.tile_pool(name="p_ctx2", bufs=1, space="PSUM") as p_ctx2, \
                 tc.tile_pool(name="p_kv2", bufs=3, space="PSUM") as p_kv2:
                self.p_kv, self.kep, self.knp = p_kv2, kep2, knp2
                self.vap, self.smallp = vap2, smallp2
                SWK = self.load_w512(I["sak"], w_kv2, "sak")
                SWV = self.load_w512(I["sav"], w_kv2, "sav")
                CTX2 = p_ctx2.tile([128, FT * HB], F32, tag="ctx2", name="ctx2")
                for ch in range(NCH):
                    a_ps, bo = self.ln_front(X2[ch])
                    xn4 = self.ln_apply(X2[ch], a_ps, self.xn4p, "xn4")
                    lnpack4.append((bo, xn4))
                    self.kv_ctx(xn4, bo, SWK, SWV, SASDK, SASDV, CTX2,
                                first=(ch == 0), last=(ch == NCH - 1))
                cs2 = ctxsb2.tile([128, FT * HB], F32, tag="ctxsb2",
                                  name="ctxsb2")
                nc.scalar.activation(out=cs2, in_=CTX2, func=AF.Copy,
                                     bias=0.0, scale=1.0)
                nc.sync.dma_start(out=cc2_in, in_=cs2)
            nc.gpsimd.collective_compute(
                "AllReduce", OP.add, replica_groups=GROUPS,
                ins=[cc2_in[:].opt()], outs=[cc2_out[:].opt()])

            # ============ phase C2: E2 (overlaps AllReduce) ============
            E2 = self.q_exp(X2, "saq", "fmssaq", BSAQ, lnpack4, self.epool)

            # ============ phase D: SA back + FFN2 ============
            X3 = self.attn_back(X2, E2, cc2_out, 1, I["sao"], BSAO, "sa",
                                self.resid)
            XF = self.ffn(X3, "f2w1", "f2w2", F2B1, F2B2)

            for ch in range(NCH):
                for m in range(FT):
                    nc.sync.dma_start(
                        out=self.out_t[128 * m : 128 * (m + 1),
                                       CHUNK * ch : CHUNK * (ch + 1)],
                        in_=XF[ch][m].bitcast(F32))


# ---------------------------------------------------------------------------
# host side
# ---------------------------------------------------------------------------
_PROGRAM = None
LAST_RESULTS = None


def _bf16(a):
    import ml_dtypes
    return np.ascontiguousarray(np.asarray(a, np.float32)).astype(
        ml_dtypes.bfloat16)


def _cols(v, nt):
    return np.ascontiguousarray(np.asarray(v, np.float32).reshape(nt, 128).T)


def _host_consts():
    sgbase = np.zeros((FT, 128, 24), np.float32)
    sel8 = np.zeros((FT, 8, 128), np.float32)
    for c in range(FT):
        for p in range(128):
            h = 2 * c + (1 if p >= 64 else 0)
            sgbase[c, p, h] = 1.0
            sel8[c, h, p] = 1.0
    return {
        "ones_c": np.ones((1, 128), np.float32),
        "ones_r": np.ones((128, 1), np.float32),
        "sgbase": sgbase,
        "sel8": sel8,  # converted below
    }


def _make_in_maps(inputs):
    f = lambda k: np.asarray(inputs[k], np.float32)

    def fold(w, g):
        return w * g[None, :]

    def fm_seed(wg):
        u = wg.sum(1)  # [out]
        return _bf16(u.reshape(FT, 1, 128))

    def tm_seed(wg, w, lb, b):
        u = wg.sum(1)
        bp = w @ lb + b
        return np.stack([u, bp])  # [2, C]

    wq_g = fold(f("ca_wq"), f("ln1_g"))
    saq_g = fold(f("sa_wq"), f("ln4_g"))
    sak_g = fold(f("sa_wk"), f("ln4_g"))
    sav_g = fold(f("sa_wv"), f("ln4_g"))
    wk_g = np.stack([fold(f("ca_wk")[i], f("ln2_g")[i]) for i in range(NIN)])
    wv_g = np.stack([fold(f("ca_wv")[i], f("ln2_g")[i]) for i in range(NIN)])
    f1w1_g = fold(f("ffn1_w1"), f("ln3_g"))
    f2w1_g = fold(f("ffn2_w1"), f("ln5_g"))

    kvsd = np.zeros((NIN, 2, 2, C), np.float32)
    for i in range(NIN):
        kvsd[i, 0] = tm_seed(wk_g[i], f("ca_wk")[i], f("ln2_b")[i],
                             f("ca_bk")[i])
        kvsd[i, 1] = tm_seed(wv_g[i], f("ca_wv")[i], f("ln2_b")[i],
                             f("ca_bv")[i])
    sasd = np.zeros((2, 2, C), np.float32)
    sasd[0] = tm_seed(sak_g, f("sa_wk"), f("ln4_b"), f("sa_bk"))
    sasd[1] = tm_seed(sav_g, f("sa_wv"), f("ln4_b"), f("sa_bv"))

    shared = {
        "wq": _bf16(wq_g.T),
        "wo": _bf16(f("ca_wo").T),
        "saq": _bf16(saq_g.T),
        "sao": _bf16(f("sa_wo").T),
        "sak": _bf16(sak_g.T),
        "sav": _bf16(sav_g.T),
        "wk": _bf16(wk_g.transpose(0, 2, 1)),
        "wv": _bf16(wv_g.transpose(0, 2, 1)),
        "f1w1": _bf16(f1w1_g.T),
        "f1w2": _bf16(f("ffn1_w2").T),
        "f2w1": _bf16(f2w1_g.T),
        "f2w2": _bf16(f("ffn2_w2").T),
        "fmsq": fm_seed(wq_g),
        "fmssaq": fm_seed(saq_g),
        "kvsd": _bf16(kvsd),
        "sasd": _bf16(sasd),
        "bq_c": _cols(f("ca_wq") @ f("ln1_b") + f("ca_bq"), FT),
        "bsaq_c": _cols(f("sa_wq") @ f("ln4_b") + f("sa_bq"), FT),
        "bo_c": _cols(f("ca_bo"), FT),
        "bsao_c": _cols(f("sa_bo"), FT),
        "f1b1_c": _cols(f("ffn1_w1") @ f("ln3_b") + f("ffn1_b1"), IT),
        "f2b1_c": _cols(f("ffn2_w1") @ f("ln5_b") + f("ffn2_b1"), IT),
        "f1b2_c": _cols(f("ffn1_b2"), FT),
        "f2b2_c": _cols(f("ffn2_b2"), FT),
    }
    hc = _host_consts()
    hc["sel8"] = _bf16(hc["sel8"])
    shared.update(hc)

    x = f("x")
    ys = f("ys")
    in_maps = []
    for core in range(N_CORES):
        b, half = core // 2, core % 2
        lo, hi = half * NTOK, (half + 1) * NTOK
        m = dict(shared)
        m["xT"] = np.ascontiguousarray(x[b, lo:hi, :].T)
        m["ysT"] = np.ascontiguousarray(ys[:, b, lo:hi, :].transpose(0, 2, 1))
        in_maps.append(m)
    return in_maps


def kernel(**inputs):
    global _PROGRAM, LAST_RESULTS
    if _PROGRAM is None:
        _PROGRAM = _build_program()
    nc = _PROGRAM
    in_maps = _make_in_maps(inputs)

    trace = os.environ.get("BASS_TRACE", "") not in ("", "0")
    res = run_bass_kernel_spmd(nc, in_maps, core_ids=list(range(N_CORES)),
                               trace=trace)
    LAST_RESULTS = res

    out = np.empty((B, T, C), np.float32)
    for core in range(N_CORES):
        b, half = core // 2, core % 2
        lo, hi = half * NTOK, (half + 1) * NTOK
        out[b, lo:hi, :] = res.results[core]["outT"].T
    return out


# revision 21
# speedup vs baseline: 1.4550x; 1.0276x over previous
"""Trainium2 Bass kernel for nn_CrossAttentionBlock (B=4, T=4096, C=512, H=8,
INNER=2048, NIN=2) on 8 NeuronCores.

Sharding: core c handles batch b=c//2, token half h=c%2 (2048 tokens each).
Cross-core coupling: only the linear-attention context (ctx = k^T v) and
k_sum, pair-wise AllReduced (cores 2b, 2b+1).

v2 design notes (vs the v1 feature-major baseline):
- LN gains folded into weights host-side; LN bias folded into projection
  biases. The per-token mean-shift enters each projection through a K<=2
  "seed" matmul (beta (x) u + 1 (x) b') that replaces the plain bias matmul.
- inv_std = exp(-0.5*ln(var+eps)) so LN, softmax-exp and all copies share
  ONE scalar-engine activation table (natural_log_exp family); gelu is the
  only other table -> ~4 table loads total.
- All heavy matmuls in bf16 (1 cyc/row incl. the 65-row ctx matmuls that
  were 4 cyc/row in fp32r); LN stats matmuls stay fp32r; residual fp32.
- scalar_tensor_tensor reads broadcast/projection results straight from
  PSUM (no PSUM->SBUF copy ops); squares and k-normalization run on the
  idle gpsimd (Pool) engine; reciprocals use reciprocal_approx_fast.
- attn combine: out = E/S + sum_i (E@ctx_i)/G_i is computed as
  qn + sum_i BD_i.T @ (E o bc(1/G_i)) with the per-head scale applied to E
  BEFORE the block-diag matmul (legal: scale is constant within a head),
  so the BD products accumulate in PSUM.
- ks_i emerges as an extra all-ones column in the ctx matmul, pre-laid-out
  in the [128, 260] folded DRAM tile that the AllReduce moves, so SG/BD
  assembly after the collective is a handful of plain DMAs.
- Phase order: kv/ctx first, then the collective overlaps the q/E (and E2)
  production.
"""
import os
import numpy as np

import concourse.bass as bass
import concourse.tile as tile
from concourse import mybir
from concourse.vector_clock import ScopedClock
from concourse.bass_utils import run_bass_kernel_spmd

F32 = mybir.dt.float32
F32R = mybir.dt.float32r
BF16 = mybir.dt.bfloat16
AF = mybir.ActivationFunctionType
OP = mybir.AluOpType

B, T, C, H, D, INNER, NIN = 4, 4096, 512, 8, 64, 2048, 2
N_CORES = 8
NTOK = 2048          # tokens per core
CHUNK = 512          # tokens per chunk
NCH = NTOK // CHUNK  # 4 chunks
FT = C // 128        # 4 feature tiles
IT = INNER // 128    # 16 inner tiles
HB = 65              # head block width in ctx psum (64 v cols + 1 ks col)
LN_EPS = 1e-5
GROUPS = [[0, 1], [2, 3], [4, 5], [6, 7]]

_split_counter = [0]


def _split_multi_waits(nc):
    """This walrus build only supports one sync-wait per instruction; move
    extra waits onto same-engine NoOps placed immediately before."""
    for f in nc.m.functions:
        for blk in f.blocks:
            out = []
            changed = False
            for inst in blk.instructions:
                si = inst.sync_info
                if si is not None and si.on_wait and len(si.on_wait) > 1:
                    waits = list(si.on_wait)
                    for w in waits[:-1]:
                        _split_counter[0] += 1
                        nop = mybir.InstNoOp(
                            name=f"I-waitsplit-{_split_counter[0]}", ins=[], outs=[]
                        )
                        nop.engine = inst.engine
                        nop.sync_info = mybir.SyncInfo(on_wait=[w], on_update=[])
                        out.append(nop)
                    si.on_wait = waits[-1:]
                    inst.sync_info = si
                    changed = True
                out.append(inst)
            if changed:
                blk.instructions = out


class _TC(tile.TileContext):
    def _drain_and_barrier(self, tick_clock, wait_clock):
        drain_inst = self.nc.sync.drain()
        wait_clock.add_sem_waits(
            drain_inst.ins, ScopedClock({None: tick_clock.global_clock})
        )
        si = drain_inst.ins.sync_info
        if si is not None and si.on_wait and len(si.on_wait) > 1:
            waits = list(si.on_wait)
            si.on_wait = waits[:1]
            drain_inst.ins.sync_info = si
            for i in range(1, len(waits)):
                extra = self.nc.sync.drain()
                esi = extra.ins.sync_info
                if esi is None:
                    extra.ins.sync_info = mybir.SyncInfo(
                        on_wait=waits[i : i + 1], on_update=[]
                    )
                else:
                    esi.on_wait = waits[i : i + 1]
                    extra.ins.sync_info = esi
        self.nc.all_engine_barrier()
        assert self.sems is not None
        popped = self.nc._tile_sem_poison_stack.pop()
        assert popped is self._sem_poison
        self.nc.clear_and_free_semaphores(list(self.sems.allocated().values()))
        self.nc.all_engine_barrier()


def _build_program(split=True):
    nc = bass.Bass("TRN2", target_bir_lowering=False, debug=False, num_devices=N_CORES)
    I = {}

    def di(name, shape, dt=F32):
        I[name] = nc.dram_tensor(name, list(shape), dt, kind="ExternalInput").ap()

    di("xT", [C, NTOK])
    di("ysT", [NIN, C, NTOK])
    # bf16 weights, [in, out] layout, LN gains folded where applicable
    di("wq", [C, C], BF16)
    di("wo", [C, C], BF16)
    di("saq", [C, C], BF16)
    di("sao", [C, C], BF16)
    di("sak", [C, C], BF16)
    di("sav", [C, C], BF16)
    di("wk", [NIN, C, C], BF16)
    di("wv", [NIN, C, C], BF16)
    di("f1w1", [C, INNER], BF16)
    di("f1w2", [INNER, C], BF16)
    di("f2w1", [C, INNER], BF16)
    di("f2w2", [INNER, C], BF16)
    # seeds: FM u-rows [FT, 1, 128]; TM [2, C] = [u; b'] stacks
    di("fmsq", [FT, 1, 128], BF16)
    di("fmssaq", [FT, 1, 128], BF16)
    di("kvsd", [NIN, 2, 2, C], BF16)   # [i][k/v] -> [u; b'] rows
    di("sasd", [2, 2, C], BF16)        # [k/v] -> [u; b'] rows
    # bias cols fp32 (per-partition activation biases / stt scalars)
    di("bq_c", [128, FT])
    di("bsaq_c", [128, FT])
    di("bo_c", [128, FT])
    di("bsao_c", [128, FT])
    di("f1b1_c", [128, IT])
    di("f2b1_c", [128, IT])
    di("f1b2_c", [128, FT])
    di("f2b2_c", [128, FT])
    di("ones_c", [1, 128])
    di("ones_r", [128, 1])
    di("sgbase", [FT, 128, 24])
    di("sel8", [FT, 8, 128], BF16)

    out_t = nc.dram_tensor("outT", [C, NTOK], F32, kind="ExternalOutput").ap()

    with _TC(nc) as tc:
        _Emitter(nc, tc, I, out_t).run()
    if split:
        _split_multi_waits(nc)
    from concourse.library_overlay import lower_extended_insts
    lower_extended_insts(nc)
    return nc


class _Emitter:
    def __init__(self, nc, tc, I, out_t):
        self.nc, self.tc, self.I, self.out_t = nc, tc, I, out_t

    # ---------------- layer norm front ----------------
    def ln_front(self, x_tiles, sq_engine="pool"):
        """Stats + rows for LN on fp32(r) feature-major tiles.
        Returns (A_ps [128,CHUNK] f32 PSUM broadcast of inv_std,
                 betaones [2,CHUNK] bf16 SBUF: row0=-m*inv_std, row1=1)."""
        nc = self.nc
        s_ps = self.p_stats.tile([1, CHUNK], F32, tag="stats", name="stats")
        for k in range(FT):
            nc.tensor.matmul(s_ps, self.ONESR, x_tiles[k],
                             start=(k == 0), stop=(k == FT - 1))
        xsq = []
        for k in range(FT):
            sq = self.sqp.tile([128, CHUNK], F32R, tag="xsq", name="xsq")
            if sq_engine == "pool":
                with nc.allow_low_precision(reason="fp32r feeds matmul"):
                    nc.gpsimd.tensor_tensor(out=sq, in0=x_tiles[k].bitcast(F32),
                                            in1=x_tiles[k].bitcast(F32),
                                            op=OP.mult)
            else:
                nc.scalar.activation(out=sq, in_=x_tiles[k].bitcast(F32),
                                     func=AF.Square)
            xsq.append(sq)
        q_ps = self.p_stats.tile([1, CHUNK], F32, tag="stats", name="stats")
        for k in range(FT):
            nc.tensor.matmul(q_ps, self.ONESR, xsq[k],
                             start=(k == 0), stop=(k == FT - 1))
        # rows: mrow = -mean; var = E[x^2] - mean^2; alpha = rsqrt(var+eps)
        mrow = self.rows.tile([1, CHUNK], F32, tag="rows", name="rows")
        nc.vector.tensor_scalar(out=mrow, in0=s_ps, scalar1=-1.0 / C,
                                scalar2=None, op0=OP.mult)
        m2 = self.rows.tile([1, CHUNK], F32, tag="rows", name="rows")
        nc.vector.tensor_tensor(out=m2, in0=mrow, in1=mrow, op=OP.mult)
        var = self.rows.tile([1, CHUNK], F32, tag="rows", name="rows")
        nc.vector.scalar_tensor_tensor(out=var, in0=q_ps, scalar=1.0 / C,
                                       in1=m2, op0=OP.mult, op1=OP.subtract)
        lnv = self.rows.tile([1, CHUNK], F32, tag="rows", name="rows")
        nc.scalar.activation(out=lnv, in_=var, func=AF.Ln, bias=self.EPS,
                             scale=1.0)
        alpha = self.rows.tile([1, CHUNK], F32R, tag="rows", name="rows")
        with nc.allow_low_precision(reason="fp32r feeds matmul"):
            nc.scalar.activation(out=alpha, in_=lnv, func=AF.Exp,
                                 bias=self.ZERO1, scale=-0.5)
        bo = self.bop.tile([2, CHUNK], BF16, tag="bo", name="bo")
        nc.vector.memset(bo, 1.0)
        with nc.allow_low_precision(reason="seed row"):
            nc.vector.tensor_tensor(out=bo[0:1, :], in0=mrow,
                                    in1=alpha.bitcast(F32), op=OP.mult)
        a_ps = self.p_bc.tile([128, CHUNK], F32, tag="bc", name="bc")
        nc.tensor.matmul(a_ps, self.ONESC, alpha, start=True, stop=True)
        return a_ps, bo

    def ln_apply(self, x_tiles, a_ps, pool, tag):
        """xn[k] = x[k] * bc(inv_std)  (bf16, mean-shift via seed matmuls)"""
        nc = self.nc
        outs = []
        for k in range(FT):
            xk = pool.tile([128, CHUNK], BF16, tag=tag, name=tag)
            nc.vector.scalar_tensor_tensor(
                out=xk, in0=x_tiles[k].bitcast(F32), scalar=1.0, in1=a_ps,
                op0=OP.mult, op1=OP.mult)
            outs.append(xk)
        return outs

    def load_w512(self, ap, pool, tag, width=C):
        tiles = []
        for k in range(FT):
            t = pool.tile([128, width], BF16, tag=f"{tag}{k}", name=f"{tag}{k}")
            self.nc.sync.dma_start(out=t, in_=ap[128 * k : 128 * (k + 1), :])
            tiles.append(t)
        return tiles

    # ---------------- kv + ctx pipeline (token-major) ----------------
    def kv_ctx(self, zy, bo, WK, WV, sdk, sdv, ctx_ps, first, last):
        """One (chunk, input) step: k/v proj + softmax-k + ctx accumulation.
        zy: 4 bf16 FM tiles; bo: [2,CHUNK] betaones; sdk/sdv: [2,C] moving
        seed rows; ctx_ps: [128, 4*HB] psum tile (even heads rows 0:64,
        odd heads rows 64:128)."""
        nc = self.nc
        for t in range(FT):
            kps = self.p_kv.tile([128, C], F32, tag="kv", name="kv")
            nc.tensor.matmul(kps, bo[:, 128 * t : 128 * (t + 1)], sdk,
                             start=True, stop=False)
            for k in range(FT):
                nc.tensor.matmul(kps, zy[k][:, 128 * t : 128 * (t + 1)],
                                 WK[k], start=False, stop=(k == FT - 1))
            kE = self.kep.tile([128, C], BF16, tag="kE", name="kE")
            nc.scalar.activation(out=kE, in_=kps, func=AF.Exp,
                                 bias=self.ZERO128, scale=1.0)
            ssum = self.smallp.tile([128, H], F32, tag="ssum", name="ssum")
            nc.vector.tensor_reduce(
                out=ssum, in_=kE.rearrange("p (h d) -> p h d", d=D),
                axis=mybir.AxisListType.X, op=OP.add)
            rsum = self.smallp.tile([128, H], F32, tag="rsum", name="rsum")
            nc.vector.reciprocal(out=rsum, in_=ssum)
            kn = self.knp.tile([128, C], BF16, tag="kn", name="kn")
            for h in range(H):
                nc.vector.tensor_scalar(
                    out=kn[:, D * h : D * (h + 1)],
                    in0=kE[:, D * h : D * (h + 1)],
                    scalar1=rsum[:, h : h + 1], scalar2=None, op0=OP.mult)
            vps = self.p_kv.tile([128, C], F32, tag="kv", name="kv")
            nc.tensor.matmul(vps, bo[:, 128 * t : 128 * (t + 1)], sdv,
                             start=True, stop=False)
            for k in range(FT):
                nc.tensor.matmul(vps, zy[k][:, 128 * t : 128 * (t + 1)],
                                 WV[k], start=False, stop=(k == FT - 1))
            va = self.vap.tile([128, H * HB], BF16, tag="va", name="va")
            nc.scalar.activation(
                out=va.rearrange("p (h b) -> p h b", b=HB)[:, :, 0:D],
                in_=vps.rearrange("p (h d) -> p h d", d=D),
                func=AF.Copy, bias=0.0, scale=1.0)
            nc.vector.memset(
                va.rearrange("p (h b) -> p h b", b=HB)[:, :, D : D + 1], 1.0)
            st = first and t == 0
            sp = last and t == FT - 1
            for h in range(H):
                half, c = h % 2, h // 2
                nc.tensor.matmul(
                    ctx_ps[64 * half : 64 * half + 64, HB * c : HB * (c + 1)],
                    kn[:, D * h : D * (h + 1)],
                    va[:, HB * h : HB * (h + 1)],
                    start=st, stop=sp,
                    tile_position=(0, 64 * half))

    # ---------------- attention back ----------------
    def attn_back(self, X, E, cc_out, n_in, wo_ap, bo_cols, tagp, Xnew_pool):
        """out = E/S + sum_i BD_i.T @ (E o bc(1/G_i)); then wo proj+residual."""
        nc, tc, I = self.nc, self.tc, self.I
        Xout = [[None] * FT for _ in range(NCH)]
        cc = (lambda i: cc_out[i]) if n_in > 1 else (lambda i: cc_out)
        ncols = 8 + 8 * n_in
        with tc.tile_pool(name=f"w_{tagp}", bufs=1) as w_o, \
             tc.tile_pool(name=f"as_{tagp}", bufs=1) as attn_s, \
             tc.tile_pool(name=f"ao_{tagp}", bufs=10) as aop, \
             tc.tile_pool(name=f"rec_{tagp}", bufs=4) as recp, \
             tc.tile_pool(name=f"pg_{tagp}", bufs=1, space="PSUM") as p_g, \
             tc.tile_pool(name=f"pr_{tagp}", bufs=3, space="PSUM") as p_r:
            WO = self.load_w512(wo_ap, w_o, "wo")
            # SG tiles: base pattern + ks columns from cc_out, then -> bf16
            SGT = []
            for c in range(FT):
                sgf = attn_s.tile([128, ncols], F32, tag=f"sgf{c}", name=f"sgf{c}")
                nc.sync.dma_start(out=sgf, in_=I["sgbase"][c][:, 0:ncols])
                for i in range(n_in):
                    col = 8 + 8 * i + 2 * c
                    nc.gpsimd.dma_start(
                        out=sgf[0:64, col : col + 1],
                        in_=cc(i)[0:64, HB * c + D : HB * c + D + 1])
                    nc.gpsimd.dma_start(
                        out=sgf[64:128, col + 1 : col + 2],
                        in_=cc(i)[64:128, HB * c + D : HB * c + D + 1])
                sg = attn_s.tile([128, ncols], BF16, tag=f"sg{c}", name=f"sg{c}")
                nc.scalar.activation(out=sg, in_=sgf, func=AF.Copy, bias=0.0,
                                     scale=1.0)
                SGT.append(sg)
            BD = [[None] * FT for _ in range(n_in)]
            for i in range(n_in):
                for c in range(FT):
                    bdf = attn_s.tile([128, 128], F32, tag=f"bdf{i}_{c}",
                                      name=f"bdf{i}_{c}")
                    nc.vector.memset(bdf, 0.0)
                    nc.gpsimd.dma_start(
                        out=bdf[0:64, 0:64],
                        in_=cc(i)[0:64, HB * c : HB * c + D])
                    nc.gpsimd.dma_start(
                        out=bdf[64:128, 64:128],
                        in_=cc(i)[64:128, HB * c : HB * c + D])
                    bd = attn_s.tile([128, 128], BF16, tag=f"bd{i}_{c}",
                                     name=f"bd{i}_{c}")
                    nc.scalar.activation(out=bd, in_=bdf, func=AF.Copy,
                                         bias=0.0, scale=1.0)
                    BD[i][c] = bd

            for ch in range(NCH):
                recs = []
                for j in range(1 + n_in):
                    gps = p_g.tile([8, CHUNK], F32, tag="gps", name="gps")
                    for c in range(FT):
                        nc.tensor.matmul(gps, SGT[c][:, 8 * j : 8 * (j + 1)],
                                         E[ch][c], start=(c == 0),
                                         stop=(c == FT - 1))
                    r = recp.tile([8, CHUNK], F32, tag="rec", name="rec")
                    nc.vector.reciprocal_approx_fast(out=r, in_=gps)
                    rb = recp.tile([8, CHUNK], BF16, tag="recb", name="recb")
                    nc.vector.tensor_scalar(out=rb, in0=r, scalar1=1.0,
                                            scalar2=None, op0=OP.mult)
                    recs.append(rb)
                outc = []
                for c in range(FT):
                    Rps = []
                    for j in range(1 + n_in):
                        rp = p_r.tile([128, CHUNK], F32, tag="R", name="R")
                        nc.tensor.matmul(rp, self.SEL8[c], recs[j],
                                         start=True, stop=True)
                        Rps.append(rp)
                    qn = aop.tile([128, CHUNK], BF16, tag="qn", name="qn")
                    nc.vector.scalar_tensor_tensor(
                        out=qn, in0=E[ch][c], scalar=1.0, in1=Rps[0],
                        op0=OP.mult, op1=OP.mult)
                    bd_ps = self.p_mm.tile([128, CHUNK], F32, tag="mm", name="mm")
                    for i in range(n_in):
                        qh = aop.tile([128, CHUNK], BF16, tag="qh", name="qh")
                        nc.vector.scalar_tensor_tensor(
                            out=qh, in0=E[ch][c], scalar=1.0, in1=Rps[1 + i],
                            op0=OP.mult, op1=OP.mult)
                        nc.tensor.matmul(bd_ps, BD[i][c], qh,
                                         start=(i == 0), stop=(i == n_in - 1))
                    ao = aop.tile([128, CHUNK], BF16, tag="ao", name="ao")
                    nc.vector.scalar_tensor_tensor(
                        out=ao, in0=qn, scalar=0.0, in1=bd_ps,
                        op0=OP.add, op1=OP.add)
                    outc.append(ao)
                for m in range(FT):
                    wps = self.p_mm.tile([128, CHUNK], F32, tag="mm", name="mm")
                    for c in range(FT):
                        nc.tensor.matmul(wps, WO[c][:, 128 * m : 128 * (m + 1)],
                                         outc[c], start=(c == 0),
                                         stop=(c == FT - 1))
                    xo = Xnew_pool.tile([128, CHUNK], F32R, tag="resid",
                                        name="resid")
                    with nc.allow_low_precision(reason="fp32r resid"):
                        nc.vector.scalar_tensor_tensor(
                            out=xo, in0=X[ch][m].bitcast(F32),
                            scalar=bo_cols[:, m : m + 1], in1=wps,
                            op0=OP.add, op1=OP.add)
                    Xout[ch][m] = xo
        return Xout

    # ---------------- FFN ----------------
    def ffn(self, Xin, w1name, w2name, B1, B2):
        nc, tc, I = self.nc, self.tc, self.I
        Xout = [[None] * FT for _ in range(NCH)]
        with tc.tile_pool(name=w1name, bufs=1) as w1p, \
             tc.tile_pool(name=w2name + "s", bufs=8) as w2p, \
             tc.tile_pool(name=w1name + "h", bufs=4) as hp, \
             tc.tile_pool(name=w1name + "x", bufs=10) as xnp, \
             tc.tile_pool(name=w1name + "b", bufs=2) as bbp, \
             tc.tile_pool(name=w1name + "p", bufs=4, space="PSUM") as p_ffn:
            W1 = []
            for k in range(FT):
                t = w1p.tile([128, INNER], BF16, tag=f"w1_{k}", name=f"w1_{k}")
                nc.sync.dma_start(
                    out=t, in_=I[w1name][128 * k : 128 * (k + 1), :])
                W1.append(t)
            for ch in range(NCH):
                a_ps, bo = self.ln_front(Xin[ch])
                b_ps = self.p_bc.tile([128, CHUNK], F32, tag="bc", name="bc")
                nc.tensor.matmul(b_ps, self.ONESCB, bo[0:1, :], start=True,
                                 stop=True)
                bsb = bbp.tile([128, CHUNK], F32, tag="bsb", name="bsb")
                nc.scalar.activation(out=bsb, in_=b_ps, func=AF.Copy,
                                     bias=0.0, scale=1.0)
                xn = []
                for k in range(FT):
                    u = xnp.tile([128, CHUNK], F32, tag="u", name="u")
                    nc.vector.scalar_tensor_tensor(
                        out=u, in0=Xin[ch][k].bitcast(F32), scalar=1.0,
                        in1=a_ps, op0=OP.mult, op1=OP.mult)
                    xk = xnp.tile([128, CHUNK], BF16, tag="xn", name="xn")
                    nc.vector.tensor_tensor(out=xk, in0=u, in1=bsb, op=OP.add)
                    xn.append(xk)
                ops = [p_ffn.tile([128, CHUNK], F32, tag="ffn", name="ffn")
                       for _ in range(FT)]
                for k in range(IT):
                    hps = self.p_mm.tile([128, CHUNK], F32, tag="mm", name="mm")
                    for c in range(FT):
                        nc.tensor.matmul(hps, W1[c][:, 128 * k : 128 * (k + 1)],
                                         xn[c], start=(c == 0),
                                         stop=(c == FT - 1))
                    h = hp.tile([128, CHUNK], BF16, tag="h", name="h")
                    nc.scalar.activation(out=h, in_=hps, func=AF.Gelu_apprx_tanh,
                                         bias=B1[:, k : k + 1], scale=1.0)
                    w2t = w2p.tile([128, C], BF16, tag="w2s", name="w2s")
                    nc.sync.dma_start(
                        out=w2t, in_=I[w2name][128 * k : 128 * (k + 1), :])
                    for m in range(FT):
                        nc.tensor.matmul(ops[m],
                                         w2t[:, 128 * m : 128 * (m + 1)], h,
                                         start=(k == 0), stop=(k == IT - 1))
                for m in range(FT):
                    xo = self.resid.tile([128, CHUNK], F32R, tag="resid",
                                         name="resid")
                    with nc.allow_low_precision(reason="fp32r resid"):
                        nc.vector.scalar_tensor_tensor(
                            out=xo, in0=Xin[ch][m].bitcast(F32),
                            scalar=B2[:, m : m + 1], in1=ops[m],
                            op0=OP.add, op1=OP.add)
                    Xout[ch][m] = xo
        return Xout

    # ---------------- q / E production (feature-major) ----------------
    def q_exp(self, Xin, wname, fmname, bcol, lnpack, Epool, sq_engine="pool"):
        """E[ch][m] = exp(Wq_g @ (x o bc(alpha)) + u (x) beta + b') for all
        chunks. lnpack: None (LN computed here per chunk) or a list of
        (bo, xn_tiles) per chunk."""
        nc, tc, I = self.nc, self.tc, self.I
        E = [[None] * FT for _ in range(NCH)]
        with tc.tile_pool(name=f"w_{wname}", bufs=1) as w_q, \
             tc.tile_pool(name=f"fms_{wname}", bufs=1) as fmsp, \
             tc.tile_pool(name=f"xn_{wname}", bufs=18) as xnp:
            WQ = self.load_w512(I[wname], w_q, "wq")
            FMS = []
            for m in range(FT):
                s = fmsp.tile([1, 128], BF16, tag=f"fms{m}", name=f"fms{m}")
                nc.sync.dma_start(out=s, in_=I[fmname][m])
                FMS.append(s)
            for ch in range(NCH):
                if lnpack is None:
                    a_ps, bo = self.ln_front(Xin[ch], sq_engine=sq_engine)
                    xn = self.ln_apply(Xin[ch], a_ps, xnp, "xn")
                else:
                    bo, xn = lnpack[ch]
                for m in range(FT):
                    ps = self.p_mm.tile([128, CHUNK], F32, tag="mm", name="mm")
                    nc.tensor.matmul(ps, FMS[m], bo[0:1, :], start=True,
                                     stop=False)
                    for k in range(FT):
                        nc.tensor.matmul(ps, WQ[k][:, 128 * m : 128 * (m + 1)],
                                         xn[k], start=False,
                                         stop=(k == FT - 1))
                    e = Epool.tile([128, CHUNK], BF16, tag="E", name="E")
                    nc.scalar.activation(out=e, in_=ps, func=AF.Exp,
                                         bias=bcol[:, m : m + 1], scale=1.0)
                    E[ch][m] = e
        return E

    # ---------------- main ----------------
    def run(self):
        nc, tc, I = self.nc, self.tc, self.I
        from contextlib import ExitStack

        with ExitStack() as ctx:
            const = ctx.enter_context(tc.tile_pool(name="const", bufs=1))
            self.resid = ctx.enter_context(tc.tile_pool(name="resid", bufs=20))
            self.epool = ctx.enter_context(tc.tile_pool(name="E", bufs=16))
            self.xn4p = ctx.enter_context(tc.tile_pool(name="xn4", bufs=16))
            self.rows = ctx.enter_context(tc.tile_pool(name="rows", bufs=10))
            self.bop = ctx.enter_context(tc.tile_pool(name="bop", bufs=10))
            self.sqp = ctx.enter_context(tc.tile_pool(name="sqp", bufs=5))
            dram = ctx.enter_context(tc.tile_pool(name="dram", bufs=1,
                                                  space="DRAM"))
            self.p_mm = ctx.enter_context(
                tc.tile_pool(name="p_mm", bufs=2, space="PSUM"))
            self.p_stats = ctx.enter_context(
                tc.tile_pool(name="p_stats", bufs=1, space="PSUM"))
            self.p_bc = ctx.enter_context(
                tc.tile_pool(name="p_bc", bufs=1, space="PSUM"))

            # ---------------- constants ----------------
            self.EPS = const.tile([1, 1], F32, tag="eps", name="eps")
            nc.vector.memset(self.EPS, LN_EPS)
            self.ZERO1 = const.tile([1, 1], F32, tag="z1", name="z1")
            nc.vector.memset(self.ZERO1, 0.0)
            self.ZERO128 = const.tile([128, 1], F32, tag="z128", name="z128")
            nc.vector.memset(self.ZERO128, 0.0)
            self.ONESC = const.tile([1, 128], F32R, tag="onesc", name="onesc")
            nc.sync.dma_start(out=self.ONESC, in_=I["ones_c"].bitcast(F32R))
            self.ONESCB = const.tile([1, 128], BF16, tag="onescb", name="onescb")
            nc.vector.memset(self.ONESCB, 1.0)
            self.ONESR = const.tile([128, 1], F32R, tag="onesr", name="onesr")
            nc.sync.dma_start(out=self.ONESR, in_=I["ones_r"].bitcast(F32R))
            self.SEL8 = []
            for c in range(FT):
                s = const.tile([8, 128], BF16, tag=f"sel8_{c}", name=f"sel8_{c}")
                nc.sync.dma_start(out=s, in_=I["sel8"][c])
                self.SEL8.append(s)

            def cols_tile(name, nt):
                t = const.tile([128, nt], F32, tag=name)
                nc.sync.dma_start(out=t, in_=I[name])
                return t

            BQ = cols_tile("bq_c", FT)
            BSAQ = cols_tile("bsaq_c", FT)
            BO = cols_tile("bo_c", FT)
            BSAO = cols_tile("bsao_c", FT)
            F1B1 = cols_tile("f1b1_c", IT)
            F1B2 = cols_tile("f1b2_c", FT)
            F2B1 = cols_tile("f2b1_c", IT)
            F2B2 = cols_tile("f2b2_c", FT)

            KVSD = []
            for i in range(NIN):
                sdk = const.tile([2, C], BF16, tag=f"sdk{i}", name=f"sdk{i}")
                nc.sync.dma_start(out=sdk, in_=I["kvsd"][i, 0])
                sdv = const.tile([2, C], BF16, tag=f"sdv{i}", name=f"sdv{i}")
                nc.sync.dma_start(out=sdv, in_=I["kvsd"][i, 1])
                KVSD.append((sdk, sdv))
            SASDK = const.tile([2, C], BF16, tag="sasdk", name="sasdk")
            nc.sync.dma_start(out=SASDK, in_=I["sasd"][0])
            SASDV = const.tile([2, C], BF16, tag="sasdv", name="sasdv")
            nc.sync.dma_start(out=SASDV, in_=I["sasd"][1])

            # ---------------- residual load ----------------
            X = [[self.resid.tile([128, CHUNK], F32R, tag="resid", name="resid")
                  for _ in range(FT)] for _ in range(NCH)]
            for ch in range(NCH):
                for c in range(FT):
                    nc.sync.dma_start(
                        out=X[ch][c],
                        in_=I["xT"][128 * c : 128 * (c + 1),
                                    CHUNK * ch : CHUNK * (ch + 1)].bitcast(F32R))

            # ============ phase A: CA kv + ctx ============
            cc_in = dram.tile([NIN, 128, FT * HB], F32, tag="cc_ca_in",
                              name="cc_ca_in")
            cc_out = dram.tile([NIN, 128, FT * HB], F32, tag="cc_ca_out",
                               name="cc_ca_out")
            with tc.tile_pool(name="w_kv", bufs=1) as w_kv, \
                 tc.tile_pool(name="ysp", bufs=12) as ysp, \
                 tc.tile_pool(name="zyp", bufs=12) as zyp, \
                 tc.tile_pool(name="kep", bufs=2) as kep, \
                 tc.tile_pool(name="knp", bufs=2) as knp, \
                 tc.tile_pool(name="vap", bufs=2) as vap, \
                 tc.tile_pool(name="smallp", bufs=4) as smallp, \
                 tc.tile_pool(name="ctxsb", bufs=2) as ctxsb, \
                 tc.tile_pool(name="p_ctx", bufs=1, space="PSUM") as p_ctx, \
                 tc.tile_pool(name="p_kv", bufs=2, space="PSUM") as p_kv:
                self.p_kv, self.kep, self.knp = p_kv, kep, knp
                self.vap, self.smallp = vap, smallp
                WK = [self.load_w512(I["wk"][i], w_kv, f"wk{i}")
                      for i in range(NIN)]
                WV = [self.load_w512(I["wv"][i], w_kv, f"wv{i}")
                      for i in range(NIN)]
                CTX = [p_ctx.tile([128, FT * HB], F32, tag=f"ctx{i}",
                                  name=f"ctx{i}") for i in range(NIN)]
                for ch in range(NCH):
                    for i in range(NIN):
                        yt = []
                        for c in range(FT):
                            y = ysp.tile([128, CHUNK], F32R, tag="ys", name="ys")
                            nc.sync.dma_start(
                                out=y,
                                in_=I["ysT"][i, 128 * c : 128 * (c + 1),
                                             CHUNK * ch : CHUNK * (ch + 1)
                                             ].bitcast(F32R))
                            yt.append(y)
                        a_ps, bo = self.ln_front(yt)
                        zy = self.ln_apply(yt, a_ps, zyp, "zy")
                        self.kv_ctx(zy, bo, WK[i], WV[i], KVSD[i][0],
                                    KVSD[i][1], CTX[i],
                                    first=(ch == 0), last=(ch == NCH - 1))
                for i in range(NIN):
                    cs = ctxsb.tile([128, FT * HB], F32, tag="ctxsb",
                                    name="ctxsb")
                    nc.scalar.activation(out=cs, in_=CTX[i], func=AF.Copy,
                                         bias=0.0, scale=1.0)
                    nc.sync.dma_start(out=cc_in[i], in_=cs)
            nc.gpsimd.collective_compute(
                "AllReduce", OP.add, replica_groups=GROUPS,
                ins=[cc_in[:].opt()], outs=[cc_out[:].opt()])

            # ============ phase A2: q/E (overlaps AllReduce) ============
            E = self.q_exp(X, "wq", "fmsq", BQ, None, self.epool,
                           sq_engine="scalar")

            # ============ phase B: CA back + FFN1 ============
            X1 = self.attn_back(X, E, cc_out, NIN, I["wo"], BO, "ca",
                                self.resid)
            X2 = self.ffn(X1, "f1w1", "f1w2", F1B1, F1B2)

            # ============ phase C: SA kv + ctx ============
            cc2_in = dram.tile([128, FT * HB], F32, tag="cc_sa_in",
                               name="cc_sa_in")
            cc2_out = dram.tile([128, FT * HB], F32, tag="cc_sa_out",
                                name="cc_sa_out")
            lnpack4 = []
            with tc.tile_pool(name="w_kv2", bufs=1) as w_kv2, \
                 tc.tile_pool(name="kep2", bufs=2) as kep2, \
                 tc.tile_pool(name="knp2", bufs=2) as knp2, \
                 tc.tile_pool(name="vap2", bufs=2) as vap2, \
                 tc.tile_pool(name="smallp2", bufs=4) as smallp2, \
                 tc.tile_pool(name="ctxsb2", bufs=1) as ctxsb2, \
                 tc.tile_pool(name="p_ctx2", bufs=1, space="PSUM") as p_ctx2, \
                 tc.tile_pool(name="p_kv2", bufs=3, space="PSUM") as p_kv2:
                self.p_kv, self.kep, self.knp = p_kv2, kep2, knp2
                self.vap, self.smallp = vap2, smallp2
                SWK = self.load_w512(I["sak"], w_kv2, "sak")
                SWV = self.load_w512(I["sav"], w_kv2, "sav")
                CTX2 = p_ctx2.tile([128, FT * HB], F32, tag="ctx2", name="ctx2")
                for ch in range(NCH):
                    a_ps, bo = self.ln_front(X2[ch], sq_engine="scalar")
                    xn4 = self.ln_apply(X2[ch], a_ps, self.xn4p, "xn4")
                    lnpack4.append((bo, xn4))
                for ch in range(NCH):
                    self.kv_ctx(lnpack4[ch][1], lnpack4[ch][0], SWK, SWV,
                                SASDK, SASDV, CTX2,
                                first=(ch == 0), last=(ch == NCH - 1))
                cs2 = ctxsb2.tile([128, FT * HB], F32, tag="ctxsb2",
                                  name="ctxsb2")
                nc.scalar.activation(out=cs2, in_=CTX2, func=AF.Copy,
                                     bias=0.0, scale=1.0)
                nc.sync.dma_start(out=cc2_in, in_=cs2)
            nc.gpsimd.collective_compute(
                "AllReduce", OP.add, replica_groups=GROUPS,
                ins=[cc2_in[:].opt()], outs=[cc2_out[:].opt()])

            # ============ phase C2: E2 (overlaps AllReduce) ============
            E2 = self.q_exp(X2, "saq", "fmssaq", BSAQ, lnpack4, self.epool)

            # ============ phase D: SA back + FFN2 ============
            X3 = self.attn_back(X2, E2, cc2_out, 1, I["sao"], BSAO, "sa",
                                self.resid)
            XF = self.ffn(X3, "f2w1", "f2w2", F2B1, F2B2)

            for ch in range(NCH):
                for m in range(FT):
                    nc.sync.dma_start(
                        out=self.out_t[128 * m : 128 * (m + 1),
                                       CHUNK * ch : CHUNK * (ch + 1)],
                        in_=XF[ch][m].bitcast(F32))


# ---------------------------------------------------------------------------
# host side
# ---------------------------------------------------------------------------
_PROGRAM = None
LAST_RESULTS = None


def _bf16(a):
    import ml_dtypes
    return np.ascontiguousarray(np.asarray(a, np.float32)).astype(
        ml_dtypes.bfloat16)


def _cols(v, nt):
    return np.ascontiguousarray(np.asarray(v, np.float32).reshape(nt, 128).T)


def _host_consts():
    sgbase = np.zeros((FT, 128, 24), np.float32)
    sel8 = np.zeros((FT, 8, 128), np.float32)
    for c in range(FT):
        for p in range(128):
            h = 2 * c + (1 if p >= 64 else 0)
            sgbase[c, p, h] = 1.0
            sel8[c, h, p] = 1.0
    return {
        "ones_c": np.ones((1, 128), np.float32),
        "ones_r": np.ones((128, 1), np.float32),
        "sgbase": sgbase,
        "sel8": sel8,  # converted below
    }


def _make_in_maps(inputs):
    f = lambda k: np.asarray(inputs[k], np.float32)

    def fold(w, g):
        return w * g[None, :]

    def fm_seed(wg):
        u = wg.sum(1)  # [out]
        return _bf16(u.reshape(FT, 1, 128))

    def tm_seed(wg, w, lb, b):
        u = wg.sum(1)
        bp = w @ lb + b
        return np.stack([u, bp])  # [2, C]

    wq_g = fold(f("ca_wq"), f("ln1_g"))
    saq_g = fold(f("sa_wq"), f("ln4_g"))
    sak_g = fold(f("sa_wk"), f("ln4_g"))
    sav_g = fold(f("sa_wv"), f("ln4_g"))
    wk_g = np.stack([fold(f("ca_wk")[i], f("ln2_g")[i]) for i in range(NIN)])
    wv_g = np.stack([fold(f("ca_wv")[i], f("ln2_g")[i]) for i in range(NIN)])
    f1w1_g = fold(f("ffn1_w1"), f("ln3_g"))
    f2w1_g = fold(f("ffn2_w1"), f("ln5_g"))

    kvsd = np.zeros((NIN, 2, 2, C), np.float32)
    for i in range(NIN):
        kvsd[i, 0] = tm_seed(wk_g[i], f("ca_wk")[i], f("ln2_b")[i],
                             f("ca_bk")[i])
        kvsd[i, 1] = tm_seed(wv_g[i], f("ca_wv")[i], f("ln2_b")[i],
                             f("ca_bv")[i])
    sasd = np.zeros((2, 2, C), np.float32)
    sasd[0] = tm_seed(sak_g, f("sa_wk"), f("ln4_b"), f("sa_bk"))
    sasd[1] = tm_seed(sav_g, f("sa_wv"), f("ln4_b"), f("sa_bv"))

    shared = {
        "wq": _bf16(wq_g.T),
        "wo": _bf16(f("ca_wo").T),
        "saq": _bf16(saq_g.T),
        "sao": _bf16(f("sa_wo").T),
        "sak": _bf16(sak_g.T),
        "sav": _bf16(sav_g.T),
        "wk": _bf16(wk_g.transpose(0, 2, 1)),
        "wv": _bf16(wv_g.transpose(0, 2, 1)),
        "f1w1": _bf16(f1w1_g.T),
        "f1w2": _bf16(f("ffn1_w2").T),
        "f2w1": _bf16(f2w1_g.T),
        "f2w2": _bf16(f("ffn2_w2").T),
        "fmsq": fm_seed(wq_g),
        "fmssaq": fm_seed(saq_g),
        "kvsd": _bf16(kvsd),
        "sasd": _bf16(sasd),
        "bq_c": _cols(f("ca_wq") @ f("ln1_b") + f("ca_bq"), FT),
        "bsaq_c": _cols(f("sa_wq") @ f("ln4_b") + f("sa_bq"), FT),
        "bo_c": _cols(f("ca_bo"), FT),
        "bsao_c": _cols(f("sa_bo"), FT),
        "f1b1_c": _cols(f("ffn1_w1") @ f("ln3_b") + f("ffn1_b1"), IT),
        "f2b1_c": _cols(f("ffn2_w1") @ f("ln5_b") + f("ffn2_b1"), IT),
        "f1b2_c": _cols(f("ffn1_b2"), FT),
        "f2b2_c": _cols(f("ffn2_b2"), FT),
    }
    hc = _host_consts()
    hc["sel8"] = _bf16(hc["sel8"])
    shared.update(hc)

    x = f("x")
    ys = f("ys")
    in_maps = []
    for core in range(N_CORES):
        b, half = core // 2, core % 2
        lo, hi = half * NTOK, (half + 1) * NTOK
        m = dict(shared)
        m["xT"] = np.ascontiguousarray(x[b, lo:hi, :].T)
        m["ysT"] = np.ascontiguousarray(ys[:, b, lo:hi, :].transpose(0, 2, 1))
        in_maps.append(m)
    return in_maps


def kernel(**inputs):
    global _PROGRAM, LAST_RESULTS
    if _PROGRAM is None:
        _PROGRAM = _build_program()
    nc = _PROGRAM
    in_maps = _make_in_maps(inputs)

    trace = os.environ.get("BASS_TRACE", "") not in ("", "0")
    res = run_bass_kernel_spmd(nc, in_maps, core_ids=list(range(N_CORES)),
                               trace=trace)
    LAST_RESULTS = res

    out = np.empty((B, T, C), np.float32)
    for core in range(N_CORES):
        b, half = core // 2, core % 2
        lo, hi = half * NTOK, (half + 1) * NTOK
        out[b, lo:hi, :] = res.results[core]["outT"].T
    return out


# revision 22
# speedup vs baseline: 1.6127x; 1.1084x over previous
"""Trainium2 Bass kernel for nn_CrossAttentionBlock (B=4, T=4096, C=512, H=8,
INNER=2048, NIN=2) on 8 NeuronCores.

Sharding: core c handles batch b=c//2, token half h=c%2 (2048 tokens each).
Cross-core coupling: only the linear-attention context (ctx = k^T v) and
k_sum, pair-wise AllReduced (cores 2b, 2b+1).

v2 design notes (vs the v1 feature-major baseline):
- LN gains folded into weights host-side; LN bias folded into projection
  biases. The per-token mean-shift enters each projection through a K<=2
  "seed" matmul (beta (x) u + 1 (x) b') that replaces the plain bias matmul.
- inv_std = exp(-0.5*ln(var+eps)) so LN, softmax-exp and all copies share
  ONE scalar-engine activation table (natural_log_exp family); gelu is the
  only other table -> ~4 table loads total.
- All heavy matmuls in bf16 (1 cyc/row incl. the 65-row ctx matmuls that
  were 4 cyc/row in fp32r); LN stats matmuls stay fp32r; residual fp32.
- scalar_tensor_tensor reads broadcast/projection results straight from
  PSUM (no PSUM->SBUF copy ops); squares and k-normalization run on the
  idle gpsimd (Pool) engine; reciprocals use reciprocal_approx_fast.
- attn combine: out = E/S + sum_i (E@ctx_i)/G_i is computed as
  qn + sum_i BD_i.T @ (E o bc(1/G_i)) with the per-head scale applied to E
  BEFORE the block-diag matmul (legal: scale is constant within a head),
  so the BD products accumulate in PSUM.
- ks_i emerges as an extra all-ones column in the ctx matmul, pre-laid-out
  in the [128, 260] folded DRAM tile that the AllReduce moves, so SG/BD
  assembly after the collective is a handful of plain DMAs.
- Phase order: kv/ctx first, then the collective overlaps the q/E (and E2)
  production.
"""
import os
import numpy as np

import concourse.bass as bass
import concourse.tile as tile
from concourse import mybir
from concourse.vector_clock import ScopedClock
from concourse.bass_utils import run_bass_kernel_spmd

F32 = mybir.dt.float32
F32R = mybir.dt.float32r
BF16 = mybir.dt.bfloat16
AF = mybir.ActivationFunctionType
OP = mybir.AluOpType

B, T, C, H, D, INNER, NIN = 4, 4096, 512, 8, 64, 2048, 2
N_CORES = 8
NTOK = 2048          # tokens per core
CHUNK = 512          # tokens per chunk
NCH = NTOK // CHUNK  # 4 chunks
FT = C // 128        # 4 feature tiles
IT = INNER // 128    # 16 inner tiles
HB = 65              # head block width in ctx psum (64 v cols + 1 ks col)
LN_EPS = 1e-5
GROUPS = [[0, 1], [2, 3], [4, 5], [6, 7]]

_split_counter = [0]


def _split_multi_waits(nc):
    """This walrus build only supports one sync-wait per instruction; move
    extra waits onto same-engine NoOps placed immediately before."""
    for f in nc.m.functions:
        for blk in f.blocks:
            out = []
            changed = False
            for inst in blk.instructions:
                si = inst.sync_info
                if si is not None and si.on_wait and len(si.on_wait) > 1:
                    waits = list(si.on_wait)
                    for w in waits[:-1]:
                        _split_counter[0] += 1
                        nop = mybir.InstNoOp(
                            name=f"I-waitsplit-{_split_counter[0]}", ins=[], outs=[]
                        )
                        nop.engine = inst.engine
                        nop.sync_info = mybir.SyncInfo(on_wait=[w], on_update=[])
                        out.append(nop)
                    si.on_wait = waits[-1:]
                    inst.sync_info = si
                    changed = True
                out.append(inst)
            if changed:
                blk.instructions = out


class _TC(tile.TileContext):
    def _drain_and_barrier(self, tick_clock, wait_clock):
        drain_inst = self.nc.sync.drain()
        wait_clock.add_sem_waits(
            drain_inst.ins, ScopedClock({None: tick_clock.global_clock})
        )
        si = drain_inst.ins.sync_info
        if si is not None and si.on_wait and len(si.on_wait) > 1:
            waits = list(si.on_wait)
            si.on_wait = waits[:1]
            drain_inst.ins.sync_info = si
            for i in range(1, len(waits)):
                extra = self.nc.sync.drain()
                esi = extra.ins.sync_info
                if esi is None:
                    extra.ins.sync_info = mybir.SyncInfo(
                        on_wait=waits[i : i + 1], on_update=[]
                    )
                else:
                    esi.on_wait = waits[i : i + 1]
                    extra.ins.sync_info = esi
        self.nc.all_engine_barrier()
        assert self.sems is not None
        popped = self.nc._tile_sem_poison_stack.pop()
        assert popped is self._sem_poison
        self.nc.clear_and_free_semaphores(list(self.sems.allocated().values()))
        self.nc.all_engine_barrier()


def _build_program(split=True):
    nc = bass.Bass("TRN2", target_bir_lowering=False, debug=False, num_devices=N_CORES)
    I = {}

    def di(name, shape, dt=F32):
        I[name] = nc.dram_tensor(name, list(shape), dt, kind="ExternalInput").ap()

    di("xT", [C, NTOK])
    di("ysT", [NIN, C, NTOK])
    # bf16 weights, [in, out] layout, LN gains folded where applicable
    di("wq", [C, C], BF16)
    di("wo", [C, C], BF16)
    di("saq", [C, C], BF16)
    di("sao", [C, C], BF16)
    di("sak", [C, C], BF16)
    di("sav", [C, C], BF16)
    di("wk", [NIN, C, C], BF16)
    di("wv", [NIN, C, C], BF16)
    di("f1w1", [C, INNER], BF16)
    di("f1w2", [INNER, C], BF16)
    di("f2w1", [C, INNER], BF16)
    di("f2w2", [INNER, C], BF16)
    # seeds: FM u-rows [FT, 1, 128]; TM [2, C] = [u; b'] stacks
    di("fmsq", [FT, 1, 128], BF16)
    di("fmssaq", [FT, 1, 128], BF16)
    di("kvsd", [NIN, 2, 2, C], BF16)   # [i][k/v] -> [u; b'] rows
    di("sasd", [2, 2, C], BF16)        # [k/v] -> [u; b'] rows
    # bias cols fp32 (per-partition activation biases / stt scalars)
    di("bq_c", [128, FT])
    di("bsaq_c", [128, FT])
    di("bo_c", [128, FT])
    di("bsao_c", [128, FT])
    di("f1b1_c", [128, IT])
    di("f2b1_c", [128, IT])
    di("f1b2_c", [128, FT])
    di("f2b2_c", [128, FT])
    di("ones_c", [1, 128])
    di("ones_r", [128, 1])
    di("sgbase", [FT, 128, 24], BF16)
    di("sel8", [FT, 8, 128], BF16)

    out_t = nc.dram_tensor("outT", [C, NTOK], F32, kind="ExternalOutput").ap()

    with _TC(nc) as tc:
        _Emitter(nc, tc, I, out_t).run()
    if split:
        _split_multi_waits(nc)
    from concourse.library_overlay import lower_extended_insts
    lower_extended_insts(nc)
    return nc


class _Emitter:
    def __init__(self, nc, tc, I, out_t):
        self.nc, self.tc, self.I, self.out_t = nc, tc, I, out_t

    # ---------------- layer norm front ----------------
    def ln_front(self, x_tiles, sq_engine="pool"):
        """Stats + rows for LN on fp32(r) feature-major tiles.
        Returns (A_ps [128,CHUNK] f32 PSUM broadcast of inv_std,
                 betaones [2,CHUNK] bf16 SBUF: row0=-m*inv_std, row1=1)."""
        nc = self.nc
        s_ps = self.p_stats.tile([1, CHUNK], F32, tag="stats", name="stats")
        for k in range(FT):
            nc.tensor.matmul(s_ps, self.ONESR, x_tiles[k],
                             start=(k == 0), stop=(k == FT - 1))
        xsq = []
        for k in range(FT):
            sq = self.sqp.tile([128, CHUNK], F32R, tag="xsq", name="xsq")
            if sq_engine == "pool":
                with nc.allow_low_precision(reason="fp32r feeds matmul"):
                    nc.gpsimd.tensor_tensor(out=sq, in0=x_tiles[k].bitcast(F32),
                                            in1=x_tiles[k].bitcast(F32),
                                            op=OP.mult)
            else:
                nc.scalar.activation(out=sq, in_=x_tiles[k].bitcast(F32),
                                     func=AF.Square)
            xsq.append(sq)
        q_ps = self.p_stats.tile([1, CHUNK], F32, tag="stats", name="stats")
        for k in range(FT):
            nc.tensor.matmul(q_ps, self.ONESR, xsq[k],
                             start=(k == 0), stop=(k == FT - 1))
        # rows: mrow = -mean; var = E[x^2] - mean^2; alpha = rsqrt(var+eps)
        mrow = self.rows.tile([1, CHUNK], F32, tag="rows", name="rows")
        nc.vector.tensor_scalar(out=mrow, in0=s_ps, scalar1=-1.0 / C,
                                scalar2=None, op0=OP.mult)
        m2 = self.rows.tile([1, CHUNK], F32, tag="rows", name="rows")
        nc.vector.tensor_tensor(out=m2, in0=mrow, in1=mrow, op=OP.mult)
        var = self.rows.tile([1, CHUNK], F32, tag="rows", name="rows")
        nc.vector.scalar_tensor_tensor(out=var, in0=q_ps, scalar=1.0 / C,
                                       in1=m2, op0=OP.mult, op1=OP.subtract)
        lnv = self.rows.tile([1, CHUNK], F32, tag="rows", name="rows")
        nc.scalar.activation(out=lnv, in_=var, func=AF.Ln, bias=self.EPS,
                             scale=1.0)
        alpha = self.rows.tile([1, CHUNK], F32R, tag="rows", name="rows")
        with nc.allow_low_precision(reason="fp32r feeds matmul"):
            nc.scalar.activation(out=alpha, in_=lnv, func=AF.Exp,
                                 bias=self.ZERO1, scale=-0.5)
        bo = self.bop.tile([2, CHUNK], BF16, tag="bo", name="bo")
        nc.vector.memset(bo, 1.0)
        with nc.allow_low_precision(reason="seed row"):
            nc.vector.tensor_tensor(out=bo[0:1, :], in0=mrow,
                                    in1=alpha.bitcast(F32), op=OP.mult)
        a_ps = self.p_bc.tile([128, CHUNK], F32, tag="bc", name="bc")
        nc.tensor.matmul(a_ps, self.ONESC, alpha, start=True, stop=True)
        return a_ps, bo

    def ln_apply(self, x_tiles, a_ps, pool, tag):
        """xn[k] = x[k] * bc(inv_std)  (bf16, mean-shift via seed matmuls)"""
        nc = self.nc
        outs = []
        for k in range(FT):
            xk = pool.tile([128, CHUNK], BF16, tag=tag, name=tag)
            nc.vector.scalar_tensor_tensor(
                out=xk, in0=x_tiles[k].bitcast(F32), scalar=1.0, in1=a_ps,
                op0=OP.mult, op1=OP.mult)
            outs.append(xk)
        return outs

    def load_w512(self, ap, pool, tag, width=C):
        tiles = []
        for k in range(FT):
            t = pool.tile([128, width], BF16, tag=f"{tag}{k}", name=f"{tag}{k}")
            self.nc.sync.dma_start(out=t, in_=ap[128 * k : 128 * (k + 1), :])
            tiles.append(t)
        return tiles

    # ---------------- kv + ctx pipeline (token-major) ----------------
    def kv_ctx(self, zy, bo, WK, WV, sdk, sdv, ctx_ps, first, last):
        """One (chunk, input) step: k/v proj + softmax-k + ctx accumulation.
        zy: 4 bf16 FM tiles; bo: [2,CHUNK] betaones; sdk/sdv: [2,C] moving
        seed rows; ctx_ps: [128, 4*HB] psum tile (even heads rows 0:64,
        odd heads rows 64:128)."""
        nc = self.nc
        for t in range(FT):
            kps = self.p_kv.tile([128, C], F32, tag="kv", name="kv")
            nc.tensor.matmul(kps, bo[:, 128 * t : 128 * (t + 1)], sdk,
                             start=True, stop=False)
            for k in range(FT):
                nc.tensor.matmul(kps, zy[k][:, 128 * t : 128 * (t + 1)],
                                 WK[k], start=False, stop=(k == FT - 1))
            kE = self.kep.tile([128, C], BF16, tag="kE", name="kE")
            nc.scalar.activation(out=kE, in_=kps, func=AF.Exp,
                                 bias=self.ZERO128, scale=1.0)
            ssum = self.smallp.tile([128, H], F32, tag="ssum", name="ssum")
            nc.vector.tensor_reduce(
                out=ssum, in_=kE.rearrange("p (h d) -> p h d", d=D),
                axis=mybir.AxisListType.X, op=OP.add)
            rsum = self.smallp.tile([128, H], F32, tag="rsum", name="rsum")
            nc.vector.reciprocal(out=rsum, in_=ssum)
            kn = self.knp.tile([128, C], BF16, tag="kn", name="kn")
            for h in range(H):
                nc.vector.tensor_scalar(
                    out=kn[:, D * h : D * (h + 1)],
                    in0=kE[:, D * h : D * (h + 1)],
                    scalar1=rsum[:, h : h + 1], scalar2=None, op0=OP.mult)
            vps = self.p_kv.tile([128, C], F32, tag="kv", name="kv")
            nc.tensor.matmul(vps, bo[:, 128 * t : 128 * (t + 1)], sdv,
                             start=True, stop=False)
            for k in range(FT):
                nc.tensor.matmul(vps, zy[k][:, 128 * t : 128 * (t + 1)],
                                 WV[k], start=False, stop=(k == FT - 1))
            va = self.vap.tile([128, H * HB], BF16, tag="va", name="va")
            nc.scalar.activation(
                out=va.rearrange("p (h b) -> p h b", b=HB)[:, :, 0:D],
                in_=vps.rearrange("p (h d) -> p h d", d=D),
                func=AF.Copy, bias=0.0, scale=1.0)
            nc.vector.memset(
                va.rearrange("p (h b) -> p h b", b=HB)[:, :, D : D + 1], 1.0)
            st = first and t == 0
            sp = last and t == FT - 1
            for h in range(H):
                half, c = h % 2, h // 2
                nc.tensor.matmul(
                    ctx_ps[64 * half : 64 * half + 64, HB * c : HB * (c + 1)],
                    kn[:, D * h : D * (h + 1)],
                    va[:, HB * h : HB * (h + 1)],
                    start=st, stop=sp,
                    tile_position=(0, 64 * half))

    # ---------------- attention back ----------------
    def attn_back(self, X, E, cc_out, n_in, wo_ap, bo_cols, tagp, Xnew_pool):
        """out = E/S + sum_i BD_i.T @ (E o bc(1/G_i)); then wo proj+residual."""
        nc, tc, I = self.nc, self.tc, self.I
        Xout = [[None] * FT for _ in range(NCH)]
        cc = (lambda i: cc_out[i]) if n_in > 1 else (lambda i: cc_out)
        ncols = 8 + 8 * n_in
        with tc.tile_pool(name=f"w_{tagp}", bufs=1) as w_o, \
             tc.tile_pool(name=f"as_{tagp}", bufs=1) as attn_s, \
             tc.tile_pool(name=f"ao_{tagp}", bufs=10) as aop, \
             tc.tile_pool(name=f"rec_{tagp}", bufs=4) as recp, \
             tc.tile_pool(name=f"pg_{tagp}", bufs=1, space="PSUM") as p_g, \
             tc.tile_pool(name=f"pr_{tagp}", bufs=3, space="PSUM") as p_r:
            WO = self.load_w512(wo_ap, w_o, "wo")
            # SG tiles: base pattern + ks columns from cc_out, then -> bf16
            SGT = []
            for c in range(FT):
                sg = attn_s.tile([128, ncols], BF16, tag=f"sg{c}", name=f"sg{c}")
                nc.sync.dma_start(out=sg, in_=I["sgbase"][c][:, 0:ncols])
                for i in range(n_in):
                    col = 8 + 8 * i + 2 * c
                    nc.gpsimd.dma_start(
                        out=sg[0:64, col : col + 1],
                        in_=cc(i)[0:64, HB * c + D : HB * c + D + 1])
                    nc.gpsimd.dma_start(
                        out=sg[64:128, col + 1 : col + 2],
                        in_=cc(i)[64:128, HB * c + D : HB * c + D + 1])
                SGT.append(sg)
            BD = [[None] * FT for _ in range(n_in)]
            for i in range(n_in):
                for c in range(FT):
                    bd = attn_s.tile([128, 128], BF16, tag=f"bd{i}_{c}",
                                     name=f"bd{i}_{c}")
                    nc.vector.memset(bd, 0.0)
                    nc.gpsimd.dma_start(
                        out=bd[0:64, 0:64],
                        in_=cc(i)[0:64, HB * c : HB * c + D])
                    nc.gpsimd.dma_start(
                        out=bd[64:128, 64:128],
                        in_=cc(i)[64:128, HB * c : HB * c + D])
                    BD[i][c] = bd

            for ch in range(NCH):
                recs = []
                for j in range(1 + n_in):
                    gps = p_g.tile([8, CHUNK], F32, tag="gps", name="gps")
                    for c in range(FT):
                        nc.tensor.matmul(gps, SGT[c][:, 8 * j : 8 * (j + 1)],
                                         E[ch][c], start=(c == 0),
                                         stop=(c == FT - 1))
                    r = recp.tile([8, CHUNK], F32, tag="rec", name="rec")
                    nc.vector.reciprocal_approx_fast(out=r, in_=gps)
                    rb = recp.tile([8, CHUNK], BF16, tag="recb", name="recb")
                    nc.vector.tensor_scalar(out=rb, in0=r, scalar1=1.0,
                                            scalar2=None, op0=OP.mult)
                    recs.append(rb)
                outc = []
                for c in range(FT):
                    Rps = []
                    for j in range(1 + n_in):
                        rp = p_r.tile([128, CHUNK], F32, tag="R", name="R")
                        nc.tensor.matmul(rp, self.SEL8[c], recs[j],
                                         start=True, stop=True)
                        Rps.append(rp)
                    qn = aop.tile([128, CHUNK], BF16, tag="qn", name="qn")
                    nc.vector.scalar_tensor_tensor(
                        out=qn, in0=E[ch][c], scalar=1.0, in1=Rps[0],
                        op0=OP.mult, op1=OP.mult)
                    bd_ps = self.p_mm.tile([128, CHUNK], F32, tag="mm", name="mm")
                    for i in range(n_in):
                        qh = aop.tile([128, CHUNK], BF16, tag="qh", name="qh")
                        nc.vector.scalar_tensor_tensor(
                            out=qh, in0=E[ch][c], scalar=1.0, in1=Rps[1 + i],
                            op0=OP.mult, op1=OP.mult)
                        nc.tensor.matmul(bd_ps, BD[i][c], qh,
                                         start=(i == 0), stop=(i == n_in - 1))
                    ao = aop.tile([128, CHUNK], BF16, tag="ao", name="ao")
                    nc.vector.scalar_tensor_tensor(
                        out=ao, in0=qn, scalar=0.0, in1=bd_ps,
                        op0=OP.add, op1=OP.add)
                    outc.append(ao)
                for m in range(FT):
                    wps = self.p_mm.tile([128, CHUNK], F32, tag="mm", name="mm")
                    for c in range(FT):
                        nc.tensor.matmul(wps, WO[c][:, 128 * m : 128 * (m + 1)],
                                         outc[c], start=(c == 0),
                                         stop=(c == FT - 1))
                    xo = Xnew_pool.tile([128, CHUNK], F32R, tag="resid",
                                        name="resid")
                    with nc.allow_low_precision(reason="fp32r resid"):
                        nc.vector.scalar_tensor_tensor(
                            out=xo, in0=X[ch][m].bitcast(F32),
                            scalar=bo_cols[:, m : m + 1], in1=wps,
                            op0=OP.add, op1=OP.add)
                    Xout[ch][m] = xo
        return Xout

    # ---------------- FFN ----------------
    def ffn(self, Xin, w1name, w2name, B1, B2):
        nc, tc, I = self.nc, self.tc, self.I
        Xout = [[None] * FT for _ in range(NCH)]
        with tc.tile_pool(name=w1name, bufs=1) as w1p, \
             tc.tile_pool(name=w2name + "s", bufs=8) as w2p, \
             tc.tile_pool(name=w1name + "h", bufs=4) as hp, \
             tc.tile_pool(name=w1name + "x", bufs=10) as xnp, \
             tc.tile_pool(name=w1name + "b", bufs=2) as bbp, \
             tc.tile_pool(name=w1name + "p", bufs=4, space="PSUM") as p_ffn:
            W1 = []
            for k in range(FT):
                t = w1p.tile([128, INNER], BF16, tag=f"w1_{k}", name=f"w1_{k}")
                nc.sync.dma_start(
                    out=t, in_=I[w1name][128 * k : 128 * (k + 1), :])
                W1.append(t)
            for ch in range(NCH):
                a_ps, bo = self.ln_front(Xin[ch])
                b_ps = self.p_bc.tile([128, CHUNK], F32, tag="bc", name="bc")
                nc.tensor.matmul(b_ps, self.ONESCB, bo[0:1, :], start=True,
                                 stop=True)
                bsb = bbp.tile([128, CHUNK], F32, tag="bsb", name="bsb")
                nc.scalar.activation(out=bsb, in_=b_ps, func=AF.Copy,
                                     bias=0.0, scale=1.0)
                xn = []
                for k in range(FT):
                    u = xnp.tile([128, CHUNK], F32, tag="u", name="u")
                    nc.vector.scalar_tensor_tensor(
                        out=u, in0=Xin[ch][k].bitcast(F32), scalar=1.0,
                        in1=a_ps, op0=OP.mult, op1=OP.mult)
                    xk = xnp.tile([128, CHUNK], BF16, tag="xn", name="xn")
                    nc.vector.tensor_tensor(out=xk, in0=u, in1=bsb, op=OP.add)
                    xn.append(xk)
                ops = [p_ffn.tile([128, CHUNK], F32, tag="ffn", name="ffn")
                       for _ in range(FT)]
                for k in range(IT):
                    hps = self.p_mm.tile([128, CHUNK], F32, tag="mm", name="mm")
                    for c in range(FT):
                        nc.tensor.matmul(hps, W1[c][:, 128 * k : 128 * (k + 1)],
                                         xn[c], start=(c == 0),
                                         stop=(c == FT - 1))
                    h = hp.tile([128, CHUNK], BF16, tag="h", name="h")
                    nc.scalar.activation(out=h, in_=hps, func=AF.Gelu_apprx_tanh,
                                         bias=B1[:, k : k + 1], scale=1.0)
                    w2t = w2p.tile([128, C], BF16, tag="w2s", name="w2s")
                    nc.sync.dma_start(
                        out=w2t, in_=I[w2name][128 * k : 128 * (k + 1), :])
                    for m in range(FT):
                        nc.tensor.matmul(ops[m],
                                         w2t[:, 128 * m : 128 * (m + 1)], h,
                                         start=(k == 0), stop=(k == IT - 1))
                for m in range(FT):
                    xo = self.resid.tile([128, CHUNK], F32R, tag="resid",
                                         name="resid")
                    with nc.allow_low_precision(reason="fp32r resid"):
                        nc.vector.scalar_tensor_tensor(
                            out=xo, in0=Xin[ch][m].bitcast(F32),
                            scalar=B2[:, m : m + 1], in1=ops[m],
                            op0=OP.add, op1=OP.add)
                    Xout[ch][m] = xo
        return Xout

    # ---------------- q / E production (feature-major) ----------------
    def q_exp(self, Xin, wname, fmname, bcol, lnpack, Epool, sq_engine="pool"):
        """E[ch][m] = exp(Wq_g @ (x o bc(alpha)) + u (x) beta + b') for all
        chunks. lnpack: None (LN computed here per chunk) or a list of
        (bo, xn_tiles) per chunk."""
        nc, tc, I = self.nc, self.tc, self.I
        E = [[None] * FT for _ in range(NCH)]
        with tc.tile_pool(name=f"w_{wname}", bufs=1) as w_q, \
             tc.tile_pool(name=f"fms_{wname}", bufs=1) as fmsp, \
             tc.tile_pool(name=f"xn_{wname}", bufs=18) as xnp:
            WQ = self.load_w512(I[wname], w_q, "wq")
            FMS = []
            for m in range(FT):
                s = fmsp.tile([1, 128], BF16, tag=f"fms{m}", name=f"fms{m}")
                nc.sync.dma_start(out=s, in_=I[fmname][m])
                FMS.append(s)
            for ch in range(NCH):
                if lnpack is None:
                    a_ps, bo = self.ln_front(Xin[ch], sq_engine=sq_engine)
                    xn = self.ln_apply(Xin[ch], a_ps, xnp, "xn")
                else:
                    bo, xn = lnpack[ch]
                for m in range(FT):
                    ps = self.p_mm.tile([128, CHUNK], F32, tag="mm", name="mm")
                    nc.tensor.matmul(ps, FMS[m], bo[0:1, :], start=True,
                                     stop=False)
                    for k in range(FT):
                        nc.tensor.matmul(ps, WQ[k][:, 128 * m : 128 * (m + 1)],
                                         xn[k], start=False,
                                         stop=(k == FT - 1))
                    e = Epool.tile([128, CHUNK], BF16, tag="E", name="E")
                    nc.scalar.activation(out=e, in_=ps, func=AF.Exp,
                                         bias=bcol[:, m : m + 1], scale=1.0)
                    E[ch][m] = e
        return E

    # ---------------- main ----------------
    def run(self):
        nc, tc, I = self.nc, self.tc, self.I
        from contextlib import ExitStack

        with ExitStack() as ctx:
            const = ctx.enter_context(tc.tile_pool(name="const", bufs=1))
            self.resid = ctx.enter_context(tc.tile_pool(name="resid", bufs=20))
            self.epool = ctx.enter_context(tc.tile_pool(name="E", bufs=16))
            self.xn4p = ctx.enter_context(tc.tile_pool(name="xn4", bufs=16))
            self.rows = ctx.enter_context(tc.tile_pool(name="rows", bufs=10))
            self.bop = ctx.enter_context(tc.tile_pool(name="bop", bufs=10))
            self.sqp = ctx.enter_context(tc.tile_pool(name="sqp", bufs=5))
            dram = ctx.enter_context(tc.tile_pool(name="dram", bufs=1,
                                                  space="DRAM"))
            self.p_mm = ctx.enter_context(
                tc.tile_pool(name="p_mm", bufs=2, space="PSUM"))
            self.p_stats = ctx.enter_context(
                tc.tile_pool(name="p_stats", bufs=1, space="PSUM"))
            self.p_bc = ctx.enter_context(
                tc.tile_pool(name="p_bc", bufs=1, space="PSUM"))

            # ---------------- constants ----------------
            self.EPS = const.tile([1, 1], F32, tag="eps", name="eps")
            nc.vector.memset(self.EPS, LN_EPS)
            self.ZERO1 = const.tile([1, 1], F32, tag="z1", name="z1")
            nc.vector.memset(self.ZERO1, 0.0)
            self.ZERO128 = const.tile([128, 1], F32, tag="z128", name="z128")
            nc.vector.memset(self.ZERO128, 0.0)
            self.ONESC = const.tile([1, 128], F32R, tag="onesc", name="onesc")
            nc.sync.dma_start(out=self.ONESC, in_=I["ones_c"].bitcast(F32R))
            self.ONESCB = const.tile([1, 128], BF16, tag="onescb", name="onescb")
            nc.vector.memset(self.ONESCB, 1.0)
            self.ONESR = const.tile([128, 1], F32R, tag="onesr", name="onesr")
            nc.sync.dma_start(out=self.ONESR, in_=I["ones_r"].bitcast(F32R))
            self.SEL8 = []
            for c in range(FT):
                s = const.tile([8, 128], BF16, tag=f"sel8_{c}", name=f"sel8_{c}")
                nc.sync.dma_start(out=s, in_=I["sel8"][c])
                self.SEL8.append(s)

            def cols_tile(name, nt):
                t = const.tile([128, nt], F32, tag=name)
                nc.sync.dma_start(out=t, in_=I[name])
                return t

            BQ = cols_tile("bq_c", FT)
            BSAQ = cols_tile("bsaq_c", FT)
            BO = cols_tile("bo_c", FT)
            BSAO = cols_tile("bsao_c", FT)
            F1B1 = cols_tile("f1b1_c", IT)
            F1B2 = cols_tile("f1b2_c", FT)
            F2B1 = cols_tile("f2b1_c", IT)
            F2B2 = cols_tile("f2b2_c", FT)

            KVSD = []
            for i in range(NIN):
                sdk = const.tile([2, C], BF16, tag=f"sdk{i}", name=f"sdk{i}")
                nc.sync.dma_start(out=sdk, in_=I["kvsd"][i, 0])
                sdv = const.tile([2, C], BF16, tag=f"sdv{i}", name=f"sdv{i}")
                nc.sync.dma_start(out=sdv, in_=I["kvsd"][i, 1])
                KVSD.append((sdk, sdv))
            SASDK = const.tile([2, C], BF16, tag="sasdk", name="sasdk")
            nc.sync.dma_start(out=SASDK, in_=I["sasd"][0])
            SASDV = const.tile([2, C], BF16, tag="sasdv", name="sasdv")
            nc.sync.dma_start(out=SASDV, in_=I["sasd"][1])

            # ---------------- residual load ----------------
            X = [[self.resid.tile([128, CHUNK], F32R, tag="resid", name="resid")
                  for _ in range(FT)] for _ in range(NCH)]
            for ch in range(NCH):
                for c in range(FT):
                    nc.sync.dma_start(
                        out=X[ch][c],
                        in_=I["xT"][128 * c : 128 * (c + 1),
                                    CHUNK * ch : CHUNK * (ch + 1)].bitcast(F32R))

            # ============ phase A: CA kv + ctx ============
            cc_in = dram.tile([NIN, 128, FT * HB], BF16, tag="cc_ca_in",
                              name="cc_ca_in")
            cc_out = dram.tile([NIN, 128, FT * HB], BF16, tag="cc_ca_out",
                               name="cc_ca_out")
            with tc.tile_pool(name="w_kv", bufs=1) as w_kv, \
                 tc.tile_pool(name="ysp", bufs=12) as ysp, \
                 tc.tile_pool(name="zyp", bufs=12) as zyp, \
                 tc.tile_pool(name="kep", bufs=2) as kep, \
                 tc.tile_pool(name="knp", bufs=2) as knp, \
                 tc.tile_pool(name="vap", bufs=2) as vap, \
                 tc.tile_pool(name="smallp", bufs=4) as smallp, \
                 tc.tile_pool(name="ctxsb", bufs=2) as ctxsb, \
                 tc.tile_pool(name="p_ctx", bufs=1, space="PSUM") as p_ctx, \
                 tc.tile_pool(name="p_kv", bufs=2, space="PSUM") as p_kv:
                self.p_kv, self.kep, self.knp = p_kv, kep, knp
                self.vap, self.smallp = vap, smallp
                pre_y = {}
                for i in range(NIN):
                    tiles = []
                    for c in range(FT):
                        y = ysp.tile([128, CHUNK], F32R, tag="ys", name="ys")
                        nc.sync.dma_start(
                            out=y, in_=I["ysT"][i, 128 * c : 128 * (c + 1),
                                                0:CHUNK].bitcast(F32R))
                        tiles.append(y)
                    pre_y[i] = tiles
                WK = [self.load_w512(I["wk"][i], w_kv, f"wk{i}")
                      for i in range(NIN)]
                WV = [self.load_w512(I["wv"][i], w_kv, f"wv{i}")
                      for i in range(NIN)]
                CTX = [p_ctx.tile([128, FT * HB], F32, tag=f"ctx{i}",
                                  name=f"ctx{i}") for i in range(NIN)]
                for ch in range(NCH):
                    for i in range(NIN):
                        if ch == 0:
                            yt = pre_y[i]
                        else:
                            yt = []
                            for c in range(FT):
                                y = ysp.tile([128, CHUNK], F32R, tag="ys",
                                             name="ys")
                                nc.sync.dma_start(
                                    out=y,
                                    in_=I["ysT"][i, 128 * c : 128 * (c + 1),
                                                 CHUNK * ch : CHUNK * (ch + 1)
                                                 ].bitcast(F32R))
                                yt.append(y)
                        a_ps, bo = self.ln_front(yt)
                        zy = self.ln_apply(yt, a_ps, zyp, "zy")
                        self.kv_ctx(zy, bo, WK[i], WV[i], KVSD[i][0],
                                    KVSD[i][1], CTX[i],
                                    first=(ch == 0), last=(ch == NCH - 1))
                for i in range(NIN):
                    cs = ctxsb.tile([128, FT * HB], BF16, tag="ctxsb",
                                    name="ctxsb")
                    nc.scalar.activation(out=cs, in_=CTX[i], func=AF.Copy,
                                         bias=0.0, scale=1.0)
                    nc.sync.dma_start(out=cc_in[i], in_=cs)
            nc.gpsimd.collective_compute(
                "AllReduce", OP.add, replica_groups=GROUPS,
                ins=[cc_in[:].opt()], outs=[cc_out[:].opt()])

            # ============ phase A2: q/E (overlaps AllReduce) ============
            E = self.q_exp(X, "wq", "fmsq", BQ, None, self.epool,
                           sq_engine="scalar")

            # ============ phase B: CA back + FFN1 ============
            X1 = self.attn_back(X, E, cc_out, NIN, I["wo"], BO, "ca",
                                self.resid)
            X2 = self.ffn(X1, "f1w1", "f1w2", F1B1, F1B2)

            # ============ phase C: SA kv + ctx ============
            cc2_in = dram.tile([128, FT * HB], BF16, tag="cc_sa_in",
                               name="cc_sa_in")
            cc2_out = dram.tile([128, FT * HB], BF16, tag="cc_sa_out",
                                name="cc_sa_out")
            lnpack4 = []
            with tc.tile_pool(name="w_kv2", bufs=1) as w_kv2, \
                 tc.tile_pool(name="kep2", bufs=2) as kep2, \
                 tc.tile_pool(name="knp2", bufs=2) as knp2, \
                 tc.tile_pool(name="vap2", bufs=2) as vap2, \
                 tc.tile_pool(name="smallp2", bufs=4) as smallp2, \
                 tc.tile_pool(name="ctxsb2", bufs=1) as ctxsb2, \
                 tc.tile_pool(name="p_ctx2", bufs=1, space="PSUM") as p_ctx2, \
                 tc.tile_pool(name="p_kv2", bufs=3, space="PSUM") as p_kv2:
                self.p_kv, self.kep, self.knp = p_kv2, kep2, knp2
                self.vap, self.smallp = vap2, smallp2
                SWK = self.load_w512(I["sak"], w_kv2, "sak")
                SWV = self.load_w512(I["sav"], w_kv2, "sav")
                CTX2 = p_ctx2.tile([128, FT * HB], F32, tag="ctx2", name="ctx2")
                for ch in range(NCH):
                    a_ps, bo = self.ln_front(X2[ch], sq_engine="scalar")
                    xn4 = self.ln_apply(X2[ch], a_ps, self.xn4p, "xn4")
                    lnpack4.append((bo, xn4))
                for ch in range(NCH):
                    self.kv_ctx(lnpack4[ch][1], lnpack4[ch][0], SWK, SWV,
                                SASDK, SASDV, CTX2,
                                first=(ch == 0), last=(ch == NCH - 1))
                cs2 = ctxsb2.tile([128, FT * HB], BF16, tag="ctxsb2",
                                  name="ctxsb2")
                nc.scalar.activation(out=cs2, in_=CTX2, func=AF.Copy,
                                     bias=0.0, scale=1.0)
                nc.sync.dma_start(out=cc2_in, in_=cs2)
            nc.gpsimd.collective_compute(
                "AllReduce", OP.add, replica_groups=GROUPS,
                ins=[cc2_in[:].opt()], outs=[cc2_out[:].opt()])

            # ============ phase C2: E2 (overlaps AllReduce) ============
            E2 = self.q_exp(X2, "saq", "fmssaq", BSAQ, lnpack4, self.epool)

            # ============ phase D: SA back + FFN2 ============
            X3 = self.attn_back(X2, E2, cc2_out, 1, I["sao"], BSAO, "sa",
                                self.resid)
            XF = self.ffn(X3, "f2w1", "f2w2", F2B1, F2B2)

            for ch in range(NCH):
                for m in range(FT):
                    nc.sync.dma_start(
                        out=self.out_t[128 * m : 128 * (m + 1),
                                       CHUNK * ch : CHUNK * (ch + 1)],
                        in_=XF[ch][m].bitcast(F32))


# ---------------------------------------------------------------------------
# host side
# ---------------------------------------------------------------------------
_PROGRAM = None
LAST_RESULTS = None


def _bf16(a):
    import ml_dtypes
    return np.ascontiguousarray(np.asarray(a, np.float32)).astype(
        ml_dtypes.bfloat16)


def _cols(v, nt):
    return np.ascontiguousarray(np.asarray(v, np.float32).reshape(nt, 128).T)


def _host_consts():
    sgbase = np.zeros((FT, 128, 24), np.float32)
    sel8 = np.zeros((FT, 8, 128), np.float32)
    for c in range(FT):
        for p in range(128):
            h = 2 * c + (1 if p >= 64 else 0)
            sgbase[c, p, h] = 1.0
            sel8[c, h, p] = 1.0
    return {
        "ones_c": np.ones((1, 128), np.float32),
        "ones_r": np.ones((128, 1), np.float32),
        "sgbase": sgbase,
        "sel8": sel8,  # converted below
    }


def _make_in_maps(inputs):
    f = lambda k: np.asarray(inputs[k], np.float32)

    def fold(w, g):
        return w * g[None, :]

    def fm_seed(wg):
        u = wg.sum(1)  # [out]
        return _bf16(u.reshape(FT, 1, 128))

    def tm_seed(wg, w, lb, b):
        u = wg.sum(1)
        bp = w @ lb + b
        return np.stack([u, bp])  # [2, C]

    wq_g = fold(f("ca_wq"), f("ln1_g"))
    saq_g = fold(f("sa_wq"), f("ln4_g"))
    sak_g = fold(f("sa_wk"), f("ln4_g"))
    sav_g = fold(f("sa_wv"), f("ln4_g"))
    wk_g = np.stack([fold(f("ca_wk")[i], f("ln2_g")[i]) for i in range(NIN)])
    wv_g = np.stack([fold(f("ca_wv")[i], f("ln2_g")[i]) for i in range(NIN)])
    f1w1_g = fold(f("ffn1_w1"), f("ln3_g"))
    f2w1_g = fold(f("ffn2_w1"), f("ln5_g"))

    kvsd = np.zeros((NIN, 2, 2, C), np.float32)
    for i in range(NIN):
        kvsd[i, 0] = tm_seed(wk_g[i], f("ca_wk")[i], f("ln2_b")[i],
                             f("ca_bk")[i])
        kvsd[i, 1] = tm_seed(wv_g[i], f("ca_wv")[i], f("ln2_b")[i],
                             f("ca_bv")[i])
    sasd = np.zeros((2, 2, C), np.float32)
    sasd[0] = tm_seed(sak_g, f("sa_wk"), f("ln4_b"), f("sa_bk"))
    sasd[1] = tm_seed(sav_g, f("sa_wv"), f("ln4_b"), f("sa_bv"))

    shared = {
        "wq": _bf16(wq_g.T),
        "wo": _bf16(f("ca_wo").T),
        "saq": _bf16(saq_g.T),
        "sao": _bf16(f("sa_wo").T),
        "sak": _bf16(sak_g.T),
        "sav": _bf16(sav_g.T),
        "wk": _bf16(wk_g.transpose(0, 2, 1)),
        "wv": _bf16(wv_g.transpose(0, 2, 1)),
        "f1w1": _bf16(f1w1_g.T),
        "f1w2": _bf16(f("ffn1_w2").T),
        "f2w1": _bf16(f2w1_g.T),
        "f2w2": _bf16(f("ffn2_w2").T),
        "fmsq": fm_seed(wq_g),
        "fmssaq": fm_seed(saq_g),
        "kvsd": _bf16(kvsd),
        "sasd": _bf16(sasd),
        "bq_c": _cols(f("ca_wq") @ f("ln1_b") + f("ca_bq"), FT),
        "bsaq_c": _cols(f("sa_wq") @ f("ln4_b") + f("sa_bq"), FT),
        "bo_c": _cols(f("ca_bo"), FT),
        "bsao_c": _cols(f("sa_bo"), FT),
        "f1b1_c": _cols(f("ffn1_w1") @ f("ln3_b") + f("ffn1_b1"), IT),
        "f2b1_c": _cols(f("ffn2_w1") @ f("ln5_b") + f("ffn2_b1"), IT),
        "f1b2_c": _cols(f("ffn1_b2"), FT),
        "f2b2_c": _cols(f("ffn2_b2"), FT),
    }
    hc = _host_consts()
    hc["sel8"] = _bf16(hc["sel8"])
    hc["sgbase"] = _bf16(hc["sgbase"])
    shared.update(hc)

    x = f("x")
    ys = f("ys")
    in_maps = []
    for core in range(N_CORES):
        b, half = core // 2, core % 2
        lo, hi = half * NTOK, (half + 1) * NTOK
        m = dict(shared)
        m["xT"] = np.ascontiguousarray(x[b, lo:hi, :].T)
        m["ysT"] = np.ascontiguousarray(ys[:, b, lo:hi, :].transpose(0, 2, 1))
        in_maps.append(m)
    return in_maps


def kernel(**inputs):
    global _PROGRAM, LAST_RESULTS
    if _PROGRAM is None:
        _PROGRAM = _build_program()
    nc = _PROGRAM
    in_maps = _make_in_maps(inputs)

    trace = os.environ.get("BASS_TRACE", "") not in ("", "0")
    res = run_bass_kernel_spmd(nc, in_maps, core_ids=list(range(N_CORES)),
                               trace=trace)
    LAST_RESULTS = res

    out = np.empty((B, T, C), np.float32)
    for core in range(N_CORES):
        b, half = core // 2, core % 2
        lo, hi = half * NTOK, (half + 1) * NTOK
        out[b, lo:hi, :] = res.results[core]["outT"].T
    return out
